# revision 1
# baseline (speedup 1.0000x reference)
"""Trainium2 Bass kernel for nn_EnhancedHybridModel.

Pipeline per core (pure data parallel over batch, 128 images/core):
  conv1(3->32,3x3,p1)+BN+ReLU -> maxpool2 -> conv2(32->64)+BN+ReLU -> maxpool2
  -> conv3(64->128)+BN+ReLU -> avgpool2 -> fc 2048->512 -> fc 512->16
  -> softmax -> 4-qubit statevector sim (collapses to two fixed real 16x16
  matmuls built on host from q_weights) -> head 4->128->100.

Conv strategy: channels on partitions, im2col K-packing, dy via free-dim
shifted accumulating matmuls, images column-packed into the PE array via
tile_position so pooling runs on up to 128 DVE lanes.  Conv/fc1 matmul
operands are fp16 (1 cycle/row on the PE, fp32 PSUM accumulate); everything
from fc2 on is exact fp32.  The quantum layer is data-independent given
q_weights, so it folds into two real 16x16 matmuls + one reciprocal (U is
unitary, so the L2 normalization needs no sqrt).
"""

import numpy as np

NB = 128          # images per core
NCORES = 8
HP1 = 34          # padded conv1 input grid
ROWL = 1160       # padded im2col row length per image (1156 + slack)
EPS = 1e-5

_cache = {}


# ---------------------------------------------------------------------------
# host-side math (quantum layer constants, weight folding, im2col rows)
# ---------------------------------------------------------------------------

def _cnot_ring_matrix():
    M = np.zeros((16, 16), dtype=np.complex64)
    for b in range(16):
        bb = b
        for cw, tw in [(0, 1), (1, 2), (2, 3), (3, 0)]:
            if (bb >> (3 - cw)) & 1:
                bb ^= 1 << (3 - tw)
        M[bb, b] = 1.0
    return M


def _zsigns():
    return np.array([[1.0 - 2.0 * ((b >> (3 - w)) & 1) for b in range(16)]
                     for w in range(4)], dtype=np.float32)


def _quantum_unitary(q_weights):
    CN = _cnot_ring_matrix()
    U_tot = np.eye(16, dtype=np.complex64)
    for l in range(2):
        c = np.cos(q_weights[l] * 0.5).astype(np.complex64)
        s = np.sin(q_weights[l] * 0.5).astype(np.complex64)
        U = np.ones((1, 1), dtype=np.complex64)
        for q in range(4):
            g = np.array([[c[q], -1j * s[q]], [-1j * s[q], c[q]]], dtype=np.complex64)
            U = np.kron(U, g)
        U_tot = (CN @ U) @ U_tot
    return U_tot  # psi_out = psi_in @ U_tot.T


def _host_weights(inp):
    f32, f16 = np.float32, np.float16
    sc = f32(1.0 / np.sqrt(1.0 + EPS))
    out = {}

    g1 = inp['bn1_g'] * sc
    w1 = np.zeros((27, 32), f32)
    for dy in range(3):
        for dx in range(3):
            for ci in range(3):
                w1[(dy * 3 + dx) * 3 + ci, :] = inp['conv1_w'][:, ci, dy, dx] * g1
    w1r = np.zeros((128, 32), f16)
    for blk in range(4):
        w1r[32 * blk:32 * blk + 27] = w1
    out['W1R'] = w1r
    b1 = inp['conv1_b'] * g1 + inp['bn1_b']
    out['B1R'] = np.tile(b1, 4)[:, None].astype(f32)

    g2 = inp['bn2_g'] * sc
    w2 = np.zeros((96, 192), f32)
    for dy in range(3):
        for dx in range(3):
            for ci in range(32):
                w2[dx * 32 + ci, dy * 64:(dy + 1) * 64] = inp['conv2_w'][:, ci, dy, dx] * g2
    out['W2'] = w2.astype(f16)
    out['B2'] = (inp['conv2_b'] * g2 + inp['bn2_b'])[:, None].astype(f32)

    g3 = inp['bn3_g'] * sc
    w3a = np.zeros((128, 384), f32)
    w3b = np.zeros((64, 384), f32)
    for dy in range(3):
        for ci in range(64):
            for dx in range(2):
                w3a[dx * 64 + ci, dy * 128:(dy + 1) * 128] = inp['conv3_w'][:, ci, dy, dx] * g3
            w3b[ci, dy * 128:(dy + 1) * 128] = inp['conv3_w'][:, ci, dy, 2] * g3
    out['W3A'] = w3a.astype(f16)
    out['W3B'] = w3b.astype(f16)
    out['B3'] = (inp['conv3_b'] * g3 + inp['bn3_b']).astype(f32)[:, None]

    # fc1 with avgpool folded in: input index = c*16 + (y2*4+x2), pool = 0.25*sum
    fr1 = inp['fr1_w'].reshape(512, 128, 16)  # [m, c, s]
    w1fc = np.zeros((128, 16 * 512), f32)
    for s in range(16):
        w1fc[:, s * 512:(s + 1) * 512] = (fr1[:, :, s].T * 0.25)
    out['W1FC'] = w1fc.astype(f16)
    out['B1FC'] = inp['fr1_b'].astype(f32)[None, :]

    fr2 = inp['fr2_w']  # [16, 512]
    w2fc = np.zeros((128, 64), f32)
    for t in range(4):
        w2fc[:, t * 16:(t + 1) * 16] = fr2[:, t * 128:(t + 1) * 128].T
    out['W2FC'] = w2fc
    out['B2FC'] = inp['fr2_b'].astype(f32)[:, None]

    U = _quantum_unitary(np.asarray(inp['q_weights'], np.float64))
    out['URT'] = np.ascontiguousarray(np.real(U).T.astype(f32))   # [i, j] = Re(U)[j, i]
    out['UIT'] = np.ascontiguousarray(np.imag(U).T.astype(f32))

    ZS = _zsigns()
    out['WH'] = np.ascontiguousarray((inp['h1_w'] @ ZS).T.astype(f32))  # [16j, 128m]
    ah = inp['bnh_g'] * sc
    out['AH'] = ah.astype(f32)[:, None]
    out['CH'] = (ah * inp['h1_b'] + inp['bnh_b']).astype(f32)[:, None]

    out['H2WT'] = np.ascontiguousarray(inp['h2_w'].T.astype(f32))  # [128, 100]
    out['H2B'] = inp['h2_b'].astype(f32)[None, :]
    return out


def _build_xr(x):
    """Host im2col rows for conv1, packed as [128, G8*2*ROWL]: partition
    32*blk + r holds row r=(dy*3+dx)*3+ci of images (8g + 2blk + {0,1}) at
    columns (g, w).  One clean 2D DMA loads a whole 8-image chunk."""
    B = x.shape[0]
    G8 = B // 8
    xp = np.zeros((B, 3, HP1 * HP1 + 3 * HP1), np.float16)  # flat plane + slack
    xpv = xp[:, :, :HP1 * HP1].reshape(B, 3, HP1, HP1)
    xpv[:, :, 1:33, 1:33] = x
    xr = np.zeros((27, B, ROWL), np.float16)
    for dy in range(3):
        for dx in range(3):
            sh = dy * HP1 + dx
            for ci in range(3):
                xr[(dy * 3 + dx) * 3 + ci, :, :] = xp[:, ci, sh:sh + ROWL]
    # [27, B, ROWL] -> [27, G8, blk4, w2, ROWL] -> [blk4, 27, G8, w2, ROWL]
    xrb = xr.reshape(27, G8, 4, 2, ROWL).transpose(2, 0, 1, 3, 4)
    xr2 = np.zeros((4, 32, G8, 2, ROWL), np.float16)
    xr2[:, :27] = xrb
    return np.ascontiguousarray(xr2.reshape(128, G8 * 2 * ROWL))


# ---------------------------------------------------------------------------
# device program
# ---------------------------------------------------------------------------

def _build_program(nb):
    import concourse.bass as bass
    import concourse.tile as tile
    from concourse import bacc, mybir
    from concourse.masks import make_identity
    from contextlib import ExitStack

    f32 = mybir.dt.float32
    f16 = mybir.dt.float16
    AF = mybir.ActivationFunctionType
    ALU = mybir.AluOpType
    AX = mybir.AxisListType

    def view(base_ap, part_start, nparts, free_off, free_dims):
        pitch = base_ap.ap[0][0]
        return bass.AP(tensor=base_ap.tensor,
                       offset=base_ap.offset + part_start * pitch + free_off,
                       ap=[[pitch, nparts]] + [list(d) for d in free_dims])

    nc = bacc.Bacc("TRN2", target_bir_lowering=False)
    G8 = nb // 8

    XR = nc.declare_dram_parameter("xr", [128, (nb // 8) * 2 * ROWL], f16, isOutput=False)
    dparams = {}
    for name, shape, dt in [("W1R", [128, 32], f16), ("B1R", [128, 1], f32),
                            ("W2", [96, 192], f16), ("B2", [64, 1], f32),
                            ("W3A", [128, 384], f16), ("W3B", [64, 384], f16),
                            ("B3", [128, 1], f32),
                            ("W1FC", [128, 16 * 512], f16), ("B1FC", [1, 512], f32),
                            ("W2FC", [128, 64], f32), ("B2FC", [16, 1], f32),
                            ("URT", [16, 16], f32), ("UIT", [16, 16], f32),
                            ("WH", [16, 128], f32), ("AH", [128, 1], f32),
                            ("CH", [128, 1], f32),
                            ("H2WT", [128, 100], f32), ("H2B", [1, 100], f32)]:
        dparams[name] = nc.declare_dram_parameter(name, shape, dt, isOutput=False)
    OUT = nc.declare_dram_parameter("out", [nb, 100], f32, isOutput=True)

    with tile.TileContext(nc) as tc, ExitStack() as ctx:
        const = ctx.enter_context(tc.tile_pool(name="const", bufs=1))
        ct = {}
        for name in dparams:
            t = const.tile(list(dparams[name].shape), dparams[name].dtype, tag="c_" + name)
            nc.sync.dma_start(t[:], dparams[name][:])
            ct[name] = t
        ident = const.tile([128, 128], f32, tag="ident")
        make_identity(nc, ident[:])
        ones_r = const.tile([1, 128], f32, tag="ones_r")
        nc.vector.memset(ones_r[:], 1.0)
        ones_c = const.tile([16, 1], f32, tag="ones_c")
        nc.vector.memset(ones_c[:], 1.0)
        ef = const.tile([128, nb * 16], f16, tag="ef")     # fc1 input accumulator

        xrap = XR[:]

        with tc.tile_pool(name="r1p", bufs=3) as r1p, \
             tc.tile_pool(name="r2p", bufs=2) as r2p, \
             tc.tile_pool(name="r3p", bufs=2) as r3p, \
             tc.tile_pool(name="t1p", bufs=3) as t1p, \
             tc.tile_pool(name="ps1p", bufs=2, space="PSUM") as ps1p, \
             tc.tile_pool(name="ps2p", bufs=2, space="PSUM") as ps2p, \
             tc.tile_pool(name="ps3p", bufs=2, space="PSUM") as ps3p:

            for g in range(G8):
                # ---- R1: host-built 27-row im2col, 4 partition blocks x 2 imgs ----
                r1 = r1p.tile([128, 2 * ROWL], f16, tag="r1")
                nc.scalar.dma_start(r1[:], xrap[:, g * 2 * ROWL:(g + 1) * 2 * ROWL])

                # ---- R2 (conv2 rhs) / R3 (conv3 rhs) with zero borders ----
                r2 = r2p.tile([96, 8 * 324 + 8], f16, tag="r2")
                nc.gpsimd.memset(view(r2[:], 0, 32, 0, [[324, 8], [306, 2], [1, 18]]), 0.0)
                nc.gpsimd.memset(view(r2[:], 0, 32, 0, [[324, 8], [18, 18], [17, 2]]), 0.0)
                nc.gpsimd.memset(r2[0:32, 8 * 324:8 * 324 + 8], 0.0)
                r3 = r3p.tile([128, 800 + 8], f16, tag="r3")
                nc.gpsimd.memset(view(r3[:], 0, 64, 0, [[100, 8], [90, 2], [1, 10]]), 0.0)
                nc.gpsimd.memset(view(r3[:], 0, 64, 0, [[100, 8], [10, 10], [9, 2]]), 0.0)
                nc.gpsimd.memset(r3[0:64, 800:808], 0.0)

                # ---- conv1 + maxpool1 (4 images column-packed) ----
                for gi in range(2):
                    ps1 = ps1p.tile([128, 1024], f32, tag="ps1")
                    for s in range(4):
                        img = 4 * gi + s
                        blk, w = img // 2, img % 2
                        for h in range(2):
                            rhs = view(r1[:], 32 * blk, 27, w * ROWL + h * 16 * HP1,
                                       [[HP1, 16], [1, 32]])
                            nc.tensor.matmul(ps1[32 * s:32 * s + 32, h * 512:(h + 1) * 512],
                                             ct['W1R'][32 * blk:32 * blk + 27, :], rhs,
                                             start=True, stop=True,
                                             tile_position=(32 * blk, 32 * s))
                    t1 = t1p.tile([128, 256], f16, tag="t1")
                    nc.vector.tensor_reduce(
                        out=t1[:],
                        in_=view(ps1[:], 0, 128, 0, [[64, 16], [2, 16], [32, 2], [1, 2]]),
                        op=ALU.max, axis=AX.XY)
                    for s in range(4):
                        img = 4 * gi + s
                        dstv = view(r2[:], 0, 32, img * 324 + 19, [[18, 16], [1, 16]])
                        srcv = view(t1[:], 32 * s, 32, 0, [[16, 16], [1, 16]])
                        eng = nc.vector if s % 2 == 0 else nc.gpsimd
                        eng.tensor_scalar(dstv, srcv, ct['B1R'][32 * s:32 * s + 32, :],
                                          0.0, op0=ALU.add, op1=ALU.max)

                # dx-shifted copies for conv2's K-packing
                nc.gpsimd.dma_start(r2[32:64, 0:8 * 324],
                                    view(r2[:], 0, 32, 1, [[1, 8 * 324]]))
                nc.sync.dma_start(r2[64:96, 0:8 * 324],
                                  view(r2[:], 0, 32, 2, [[1, 8 * 324]]))

                # ---- conv2 + maxpool2 (2 images column-packed) ----
                for p in range(4):
                    ps2 = ps2p.tile([128, 256], f32, tag="ps2")
                    for sp in range(2):
                        img = 2 * p + sp
                        for dy in range(3):
                            rhs = view(r2[:], 0, 96, img * 324 + dy * 18,
                                       [[18, 16], [1, 16]])
                            nc.tensor.matmul(ps2[64 * sp:64 * sp + 64, :],
                                             ct['W2'][:, dy * 64:(dy + 1) * 64], rhs,
                                             start=(dy == 0), stop=(dy == 2),
                                             tile_position=(0, 64 * sp))
                    t2 = t1p.tile([128, 64], f16, tag="t2")
                    nc.vector.tensor_reduce(
                        out=t2[:],
                        in_=view(ps2[:], 0, 128, 0, [[32, 8], [2, 8], [16, 2], [1, 2]]),
                        op=ALU.max, axis=AX.XY)
                    for sp in range(2):
                        img = 2 * p + sp
                        dstv = view(r3[:], 0, 64, img * 100 + 11, [[10, 8], [1, 8]])
                        nc.gpsimd.tensor_copy(dstv, view(t2[:], 64 * sp, 64, 0, [[8, 8], [1, 8]]))

                # bias+relu on conv2 pooled interior, then dx-shift copy
                r3int = view(r3[:], 0, 64, 11, [[100, 8], [10, 8], [1, 8]])
                nc.vector.tensor_scalar(r3int, r3int, ct['B2'][:], 0.0,
                                        op0=ALU.add, op1=ALU.max)
                c3src = bass.AP(tensor=r3[:].tensor, offset=r3[:].offset + 1,
                                ap=[[r3[:].ap[0][0], 64], [1, 800]])
                nc.gpsimd.dma_start(r3[64:128, 0:800], c3src)

                # ---- conv3 + relu + avgpool -> EF ----
                ps3 = ps3p.tile([128, 512], f32, tag="ps3")
                for dy in range(3):
                    rhsA = view(r3[:], 0, 128, dy * 10, [[100, 8], [10, 8], [1, 8]])
                    nc.tensor.matmul(ps3[:], ct['W3A'][:, dy * 128:(dy + 1) * 128],
                                     rhsA, start=(dy == 0), stop=False)
                    rhsB = view(r3[:], 0, 64, dy * 10 + 2, [[100, 8], [10, 8], [1, 8]])
                    nc.tensor.matmul(ps3[:], ct['W3B'][:, dy * 128:(dy + 1) * 128],
                                     rhsB, start=False, stop=(dy == 2))
                nc.scalar.activation(ps3[:], ps3[:], AF.Relu, bias=ct['B3'][:], scale=1.0)
                with nc.allow_low_precision("avgpool sums 4 values; fp16 out is fine"):
                    nc.vector.tensor_reduce(
                        out=view(ef[:], 0, 128, g * 128, [[16, 8], [4, 4], [1, 4]]),
                        in_=view(ps3[:], 0, 128, 0, [[64, 8], [16, 4], [2, 4], [8, 2], [1, 2]]),
                        op=ALU.add, axis=AX.XY)

        # ------------------- tail: fc1 / fc2 / quantum / head -------------------
        with tc.tile_pool(name="tsb", bufs=1) as tsb, \
             tc.tile_pool(name="psfp", bufs=1, space="PSUM") as psfp, \
             tc.tile_pool(name="pstp", bufs=2, space="PSUM") as pstp, \
             tc.tile_pool(name="tp1", bufs=2, space="PSUM") as tp1, \
             tc.tile_pool(name="tp2", bufs=2, space="PSUM") as tp2:

            psf = psfp.tile([nb, 512], f32, tag="psf")
            for s in range(16):
                lhsT = view(ef[:], 0, 128, s, [[16, nb]])
                nc.tensor.matmul(psf[:], lhsT,
                                 ct['W1FC'][:, s * 512:(s + 1) * 512],
                                 start=(s == 0), stop=False)
            nc.tensor.matmul(psf[:], ones_r[0:1, 0:nb], ct['B1FC'][:],
                             start=False, stop=True)
            h1t = tsb.tile([nb, 512], f32, tag="h1t")
            nc.scalar.activation(h1t[:], psf[:], AF.Relu)

            h1 = tsb.tile([128, 4 * nb], f32, tag="h1")
            for t in range(4):
                pst = pstp.tile([128, nb], f32, tag="pst")
                nc.tensor.transpose(pst[:], h1t[:, t * 128:(t + 1) * 128], ident[0:nb, 0:nb])
                nc.scalar.copy(h1[:, t * nb:(t + 1) * nb], pst[:])

            psz = tp1.tile([16, nb], f32, tag="tp1")
            for t in range(4):
                nc.tensor.matmul(psz[:], ct['W2FC'][:, t * 16:(t + 1) * 16],
                                 h1[:, t * nb:(t + 1) * nb],
                                 start=(t == 0), stop=(t == 3))
            e = tsb.tile([16, nb], f32, tag="e")
            nc.scalar.activation(e[:], psz[:], AF.Exp, bias=ct['B2FC'][:], scale=1.0)

            psr = tp2.tile([16, nb], f32, tag="tp2")
            nc.tensor.matmul(psr[:], ct['URT'][:], e[:], start=True, stop=True)
            psi2 = tp2.tile([16, nb], f32, tag="tp2")
            nc.tensor.matmul(psi2[:], ct['UIT'][:], e[:], start=True, stop=True)
            tr = tsb.tile([16, nb], f32, tag="tr")
            nc.scalar.square(tr[:], psr[:])
            ti = tsb.tile([16, nb], f32, tag="ti")
            nc.scalar.square(ti[:], psi2[:])
            pun = tsb.tile([16, nb], f32, tag="pun")
            nc.vector.tensor_add(pun[:], tr[:], ti[:])

            pss = tp1.tile([1, nb], f32, tag="tp1")
            nc.tensor.matmul(pss[:], ones_c[:], pun[:], start=True, stop=True)
            rec = tsb.tile([1, nb], f32, tag="rec")
            nc.vector.reciprocal(rec[:], pss[:])
            psb = tp1.tile([16, nb], f32, tag="tp1")
            nc.tensor.matmul(psb[:], ones_r[0:1, 0:16], rec[:], start=True, stop=True)
            bc = tsb.tile([16, nb], f32, tag="bc")
            nc.scalar.copy(bc[:], psb[:])
            pn = tsb.tile([16, nb], f32, tag="pn")
            nc.vector.tensor_mul(pn[:], pun[:], bc[:])

            psy = tp2.tile([128, nb], f32, tag="tp2")
            nc.tensor.matmul(psy[:], ct['WH'][:], pn[:], start=True, stop=True)
            h2 = tsb.tile([128, nb], f32, tag="h2")
            nc.scalar.activation(h2[:], psy[:], AF.Relu, bias=ct['CH'][:], scale=ct['AH'][:])

            pso = tp1.tile([nb, 100], f32, tag="tp1")
            nc.tensor.matmul(pso[:], h2[:], ct['H2WT'][:], start=True, stop=False)
            nc.tensor.matmul(pso[:], ones_r[0:1, 0:nb], ct['H2B'][:],
                             start=False, stop=True)
            outs = tsb.tile([nb, 100], f32, tag="outs")
            nc.scalar.copy(outs[:], pso[:])
            nc.sync.dma_start(OUT[:], outs[:])

    nc.finalize()
    return nc


def get_program(nb=NB):
    key = ("prog", nb)
    if key not in _cache:
        _cache[key] = _build_program(nb)
    return _cache[key]


# ---------------------------------------------------------------------------
# entry point
# ---------------------------------------------------------------------------

def kernel(**inputs):
    from concourse.bass_utils import run_bass_kernel_spmd

    x = np.asarray(inputs['x'], np.float32)
    B = x.shape[0]
    nb = B // NCORES
    hw = _host_weights({k: np.asarray(v) for k, v in inputs.items()})

    nc = get_program(nb)
    in_maps = []
    for c in range(NCORES):
        m = {'xr': _build_xr(x[c * nb:(c + 1) * nb])}
        m.update(hw)
        in_maps.append(m)
    res = run_bass_kernel_spmd(nc, in_maps, core_ids=list(range(NCORES)))
    return np.concatenate([res.results[c]['out'] for c in range(NCORES)], axis=0)



# revision 20
# speedup vs baseline: 1.2305x; 1.2305x over previous
"""Trainium2 Bass kernel for nn_EnhancedHybridModel.

Pipeline per core (pure data parallel over batch, 128 images/core):
  conv1(3->32,3x3,p1)+BN+ReLU -> maxpool2 -> conv2(32->64)+BN+ReLU -> maxpool2
  -> conv3(64->128)+BN+ReLU -> avgpool2 -> fc 2048->512 -> fc 512->16
  -> softmax -> 4-qubit statevector sim (collapses to two fixed real 16x16
  matmuls built on host from q_weights) -> head 4->128->100.

Layout decisions (matmul cost scales with out free size, so every matmul
uses as many of the 128 output partitions as the layer allows):
  * conv1: 4 images block-diagonally packed on the PE (K = 4x(27+bias row),
    M = 4img x 32ch), two F=512 matmuls per 4-image unit.
  * maxpool: horizontal pairs via scalar_tensor_tensor max straight out of
    PSUM (Pool/DVE), vertical pairs via fp16 tensor_tensor max (DVE 2x mode),
    then per-image tensor_scalar writes (DVE 4x / Pool ptr-form) into the
    next conv's padded-plane tile; conv1's relu rides the first max
    (max(a,0,b)), conv2's bias+relu ride the write stage.
  * conv2: 3 dy passes, K=96 dx-preshifted planes (one shift DMA per group).
  * conv3: 6 passes (3dy x {2dx-packed 128, dx2 64}); relu+bias via one Act
    pass, avgpool via fp16 adds with the 0.25 folded into fc1 weights.
  * 2-stage software pipeline (conv1(g) runs two groups ahead of
    conv2/conv3(g)) keeps the PE dense; a dummy-matmul warmup covers the
    first input DMA so the PE p-state ramp completes early.
"""

import numpy as np

NB = 128          # images per core
NCORES = 8
ROWL = 1160       # padded im2col plane length per image (1156 + slack)
EPS = 1e-5

_cache = {}


# ---------------------------------------------------------------------------
# host-side math (quantum layer constants, weight folding, im2col planes)
# ---------------------------------------------------------------------------

def _cnot_ring_matrix():
    M = np.zeros((16, 16), dtype=np.complex64)
    for b in range(16):
        bb = b
        for cw, tw in [(0, 1), (1, 2), (2, 3), (3, 0)]:
            if (bb >> (3 - cw)) & 1:
                bb ^= 1 << (3 - tw)
        M[bb, b] = 1.0
    return M


def _zsigns():
    return np.array([[1.0 - 2.0 * ((b >> (3 - w)) & 1) for b in range(16)]
                     for w in range(4)], dtype=np.float32)


def _quantum_unitary(q_weights):
    CN = _cnot_ring_matrix()
    U_tot = np.eye(16, dtype=np.complex64)
    for l in range(2):
        c = np.cos(q_weights[l] * 0.5).astype(np.complex64)
        s = np.sin(q_weights[l] * 0.5).astype(np.complex64)
        U = np.ones((1, 1), dtype=np.complex64)
        for q in range(4):
            g = np.array([[c[q], -1j * s[q]], [-1j * s[q], c[q]]], dtype=np.complex64)
            U = np.kron(U, g)
        U_tot = (CN @ U) @ U_tot
    return U_tot  # psi_out = psi_in @ U_tot.T


def _host_weights(inp):
    f32, f16 = np.float32, np.float16
    sc = f32(1.0 / np.sqrt(1.0 + EPS))
    out = {}

    # conv1: 4-image block-diagonal [128, 128]; row 32j+27 carries the bias.
    g1 = inp['bn1_g'] * sc
    w1 = np.zeros((28, 32), f32)
    for dy in range(3):
        for dx in range(3):
            for ci in range(3):
                w1[(dy * 3 + dx) * 3 + ci, :] = inp['conv1_w'][:, ci, dy, dx] * g1
    w1[27, :] = inp['conv1_b'] * g1 + inp['bn1_b']
    w1blk = np.zeros((128, 128), f16)
    for j in range(4):
        w1blk[32 * j:32 * j + 28, 32 * j:32 * j + 32] = w1
    out['W1BLK'] = w1blk

    # conv2: [96, 3*64] rows dx*32+ci; bias applied at the pool write stage.
    g2 = inp['bn2_g'] * sc
    w2 = np.zeros((96, 192), f32)
    for dy in range(3):
        for dx in range(3):
            for ci in range(32):
                w2[dx * 32 + ci, dy * 64:(dy + 1) * 64] = inp['conv2_w'][:, ci, dy, dx] * g2
    out['W2DY'] = w2.astype(f16)
    out['B2R'] = np.tile(inp['conv2_b'] * g2 + inp['bn2_b'], 2)[:, None].astype(f32)

    # conv3: 6 passes [128, 6*128]: pass 2*dy   = chunkA (dx0,dx1 on 128 rows)
    #                               pass 2*dy+1 = chunkB (dx2 on 64 rows)
    g3 = inp['bn3_g'] * sc
    w3 = np.zeros((128, 6 * 128), f32)
    for dy in range(3):
        for ci in range(64):
            for dx in range(2):
                w3[dx * 64 + ci, (2 * dy) * 128:(2 * dy + 1) * 128] = \
                    inp['conv3_w'][:, ci, dy, dx] * g3
            w3[ci, (2 * dy + 1) * 128:(2 * dy + 2) * 128] = \
                inp['conv3_w'][:, ci, dy, 2] * g3
    out['W3P'] = w3.astype(f16)
    out['B3'] = (inp['conv3_b'] * g3 + inp['bn3_b']).astype(f32)[:, None]

    # fc1 with avgpool folded in: input index = c*16 + s, pool = 0.25*sum
    fr1 = inp['fr1_w'].reshape(512, 128, 16)  # [m, c, s]
    w1fc = np.zeros((128, 16 * 512), f32)
    for s in range(16):
        w1fc[:, s * 512:(s + 1) * 512] = (fr1[:, :, s].T * 0.25)
    out['W1FC'] = w1fc.astype(f16)
    out['B1FC'] = inp['fr1_b'].astype(f32)[None, :]

    fr2 = inp['fr2_w']  # [16, 512]
    w2fc = np.zeros((128, 64), f32)
    for t in range(4):
        w2fc[:, t * 16:(t + 1) * 16] = fr2[:, t * 128:(t + 1) * 128].T
    out['W2FC'] = w2fc
    out['B2FC'] = inp['fr2_b'].astype(f32)[:, None]

    U = _quantum_unitary(np.asarray(inp['q_weights'], np.float64))
    out['URT'] = np.ascontiguousarray(np.real(U).T.astype(f32))
    out['UIT'] = np.ascontiguousarray(np.imag(U).T.astype(f32))

    ZS = _zsigns()
    out['WH'] = np.ascontiguousarray((inp['h1_w'] @ ZS).T.astype(f32))  # [16, 128]
    ah = inp['bnh_g'] * sc
    out['AH'] = ah.astype(f32)[:, None]
    out['CH'] = (ah * inp['h1_b'] + inp['bnh_b']).astype(f32)[:, None]

    out['H2WT'] = np.ascontiguousarray(inp['h2_w'].T.astype(f32))  # [128, 100]
    out['H2B'] = inp['h2_b'].astype(f32)[None, :]
    return out


def _build_xr(x):
    """Host im2col planes for conv1, packed [128, (B//4)*ROWL]: partition
    32*j + r holds plane row r of image 4u+j at columns [u*ROWL, ...): the
    27 shifted flat padded planes, a ones row (r=27) for the bias, zeros
    above."""
    B = x.shape[0]
    U = B // 4
    xp = np.zeros((B, 3, 34 * 34 + 72), np.float16)  # flat plane + shift slack
    xpv = xp[:, :, :34 * 34].reshape(B, 3, 34, 34)
    xpv[:, :, 1:33, 1:33] = x
    xr = np.zeros((32, B, ROWL), np.float16)
    for dy in range(3):
        for dx in range(3):
            sh = dy * 34 + dx
            for ci in range(3):
                r = (dy * 3 + dx) * 3 + ci
                xr[r, :, :1156] = xp[:, ci, sh:sh + 1156]
    xr[27, :, :] = 1.0
    # [32, B, ROWL] -> [32, U, 4, ROWL] -> [4, 32, U, ROWL] -> [128, U*ROWL]
    xrb = xr.reshape(32, U, 4, ROWL).transpose(2, 0, 1, 3)
    return np.ascontiguousarray(xrb.reshape(128, U * ROWL))


# ---------------------------------------------------------------------------
# device program
# ---------------------------------------------------------------------------

def _build_program(nb):
    import concourse.bass as bass
    import concourse.tile as tile
    from concourse import bacc, mybir
    from concourse.masks import make_identity
    from contextlib import ExitStack

    f32 = mybir.dt.float32
    f16 = mybir.dt.float16
    AF = mybir.ActivationFunctionType
    ALU = mybir.AluOpType
    AX = mybir.AxisListType

    def view(base_ap, part_start, nparts, free_off, free_dims):
        pitch = base_ap.ap[0][0]
        return bass.AP(tensor=base_ap.tensor,
                       offset=base_ap.offset + part_start * pitch + free_off,
                       ap=[[pitch, nparts]] + [list(d) for d in free_dims])

    nc = bacc.Bacc("TRN2", target_bir_lowering=False)
    G = nb // 8            # groups of 8 images
    NU = nb // 4           # units of 4 images

    XR = nc.declare_dram_parameter("xr", [128, NU * ROWL], f16, isOutput=False)
    dparams = {}
    for name, shape, dt in [("W1BLK", [128, 128], f16), ("W2DY", [96, 192], f16),
                            ("B2R", [128, 1], f32),
                            ("W3P", [128, 768], f16), ("B3", [128, 1], f32),
                            ("W1FC", [128, 16 * 512], f16), ("B1FC", [1, 512], f32),
                            ("W2FC", [128, 64], f32), ("B2FC", [16, 1], f32),
                            ("URT", [16, 16], f32), ("UIT", [16, 16], f32),
                            ("WH", [16, 128], f32), ("AH", [128, 1], f32),
                            ("CH", [128, 1], f32),
                            ("H2WT", [128, 100], f32), ("H2B", [1, 100], f32)]:
        dparams[name] = nc.declare_dram_parameter(name, shape, dt, isOutput=False)
    OUT = nc.declare_dram_parameter("out", [nb, 100], f32, isOutput=True)

    R2N = 8 * 324 + 16     # 8 image planes of 18x18 + slack
    R3N = 8 * 100 + 8

    with tile.TileContext(nc) as tc, ExitStack() as ctx:
        const = ctx.enter_context(tc.tile_pool(name="const", bufs=1))
        ct = {}

        def load_param(name, eng):
            t = const.tile(list(dparams[name].shape), dparams[name].dtype,
                           tag="c_" + name)
            eng.dma_start(t[:], dparams[name][:])
            ct[name] = t

        r1p = ctx.enter_context(tc.tile_pool(name="r1p", bufs=3))
        r1t = [None] * G

        def load_r1(g):
            # two per-unit DMAs so conv1 on unit 0 starts before unit 1 lands
            r1 = r1p.tile([128, 2 * ROWL], f16, tag="r1")
            for u in range(2):
                nc.sync.dma_start(r1[:, u * ROWL:(u + 1) * ROWL],
                                  XR[:, (g * 2 + u) * ROWL:(g * 2 + u + 1) * ROWL])
            r1t[g] = r1

        # critical-path DMAs on SP, in consumption order
        load_r1(0)
        load_param("W1BLK", nc.sync)
        load_r1(1)
        for name in ["W2DY", "B2R", "W3P", "B3"]:
            load_param(name, nc.sync)

        ident = const.tile([128, 128], f32, tag="ident")
        make_identity(nc, ident[:])
        ones_r = const.tile([1, 128], f32, tag="ones_r")
        nc.vector.memset(ones_r[:], 1.0)
        ones_c = const.tile([16, 1], f32, tag="ones_c")
        nc.vector.memset(ones_c[:], 1.0)
        zp = const.tile([128, 1], f32, tag="zp")
        nc.vector.memset(zp[:], 0.0)
        ef = const.tile([128, nb * 16], f16, tag="ef")   # fc1 input accumulator

        # --- persistent triple-buffered conv input tiles with zero borders ---
        r2b, r3b = [], []
        for k in range(3):
            r2 = const.tile([128, R2N], f16, tag=f"r2_{k}")
            nc.gpsimd.memset(view(r2[:], 0, 32, 0, [[324, 8], [17 * 18, 2], [1, 18]]), 0.0)
            nc.gpsimd.memset(view(r2[:], 0, 32, 0, [[324, 8], [18, 18], [17, 2]]), 0.0)
            nc.gpsimd.memset(r2[0:32, 8 * 324:R2N], 0.0)
            r2b.append(r2)
            r3 = const.tile([128, R3N], f16, tag=f"r3_{k}")
            nc.gpsimd.memset(view(r3[:], 0, 64, 0, [[100, 8], [9 * 10, 2], [1, 10]]), 0.0)
            nc.gpsimd.memset(view(r3[:], 0, 64, 0, [[100, 8], [10, 10], [9, 2]]), 0.0)
            nc.gpsimd.memset(r3[0:64, 800:R3N], 0.0)
            r3b.append(r3)

        # --- PE p-state warmup: dummy matmuls while the first DMAs land ---
        with tc.tile_pool(name="wup", bufs=1) as wup, \
             tc.tile_pool(name="wps", bufs=1, space="PSUM") as wps:
            w0 = wup.tile([128, 512], f16, tag="w0")
            nc.vector.memset(w0[:], 0.0)
            pw = wps.tile([128, 512], f32, tag="pw")
            for _ in range(12):
                nc.tensor.matmul(pw[:], w0[:, 0:128], w0[:], start=True, stop=True)

        with tc.tile_pool(name="sxp", bufs=3) as sxp, \
             tc.tile_pool(name="ps1p", bufs=2, space="PSUM") as ps1p, \
             tc.tile_pool(name="ps2p", bufs=2, space="PSUM") as ps2p, \
             tc.tile_pool(name="ps3p", bufs=2, space="PSUM") as ps3p:

            def stage1(g):
                """conv1 + pool1 for group g (8 images, 2 units)."""
                if g + 2 < G:
                    load_r1(g + 2)
                r1 = r1t[g]
                r2 = r2b[g % 3]
                for u in range(2):
                    ps1 = ps1p.tile([128, 1024], f32, tag="ps1")
                    for h in range(2):
                        rhs = view(r1[:], 0, 128, u * ROWL + h * 16 * 34,
                                   [[34, 16], [1, 32]])
                        nc.tensor.matmul(ps1[:, h * 512:(h + 1) * 512],
                                         ct['W1BLK'][:], rhs, start=True, stop=True)
                    # pool1: unit 0 evacuates via Act relu + DVE fp16 max tree,
                    # unit 1 via a direct DVE 4:1 max reduce (relu rides the
                    # per-image write's max-with-0); both end in t1 [128, 256]
                    t1 = sxp.tile([128, 256], f16, tag="t1")
                    if u == 0:
                        s1 = sxp.tile([128, 1024], f16, tag="s1")
                        nc.scalar.activation(s1[:], ps1[:], AF.Relu)
                        th = sxp.tile([128, 512], f16, tag="th")
                        nc.vector.tensor_tensor(
                            th[:], view(s1[:], 0, 128, 0, [[32, 32], [2, 16]]),
                            view(s1[:], 0, 128, 1, [[32, 32], [2, 16]]), op=ALU.max)
                        nc.vector.tensor_tensor(
                            t1[:], view(th[:], 0, 128, 0, [[32, 16], [1, 16]]),
                            view(th[:], 0, 128, 16, [[32, 16], [1, 16]]), op=ALU.max)
                    else:
                        nc.vector.tensor_reduce(
                            out=t1[:],
                            in_=view(ps1[:], 0, 128, 0,
                                     [[64, 16], [2, 16], [32, 2], [1, 2]]),
                            op=ALU.max, axis=AX.XY)
                    for j in range(4):
                        iu = 4 * u + j
                        dstv = view(r2[:], 0, 32, iu * 324 + 19, [[18, 16], [1, 16]])
                        srcv = view(t1[:], 32 * j, 32, 0, [[16, 16], [1, 16]])
                        if j % 2 == 0:
                            nc.vector.tensor_scalar(dstv, srcv, zp[32 * j:32 * j + 32, :],
                                                    0.0, op0=ALU.add, op1=ALU.max)
                        else:
                            nc.gpsimd.tensor_scalar(dstv, srcv, zp[32 * j:32 * j + 32, :],
                                                    0.0, op0=ALU.add, op1=ALU.max)
                # dx-shift copies for conv2's K-packing (parts 32:96)
                pitch = r2[:].ap[0][0]
                for dx in (1, 2):
                    src = bass.AP(tensor=r2[:].tensor, offset=r2[:].offset + dx,
                                  ap=[[pitch, 32], [1, 8 * 324 + 4 - dx]])
                    eng = nc.sync if dx == 1 else nc.scalar
                    eng.dma_start(r2[32 * dx:32 * dx + 32, 0:8 * 324 + 4 - dx],
                                  src)

            def stage2(g):
                """conv2 + pool2 + conv3 + avgpool for group g."""
                r2 = r2b[g % 3]
                r3 = r3b[g % 3]
                for v in range(2):
                    ps2 = ps2p.tile([128, 512], f32, tag="ps2")
                    for i in range(4):            # image 4v+i within group
                        iu = 4 * v + i
                        pb, st = i % 2, i // 2    # partition block, subtile
                        for dy in range(3):
                            rhs = view(r2[:], 0, 96, iu * 324 + dy * 18,
                                       [[18, 16], [1, 16]])
                            nc.tensor.matmul(
                                ps2[64 * pb:64 * pb + 64, st * 256:st * 256 + 256],
                                ct['W2DY'][:, dy * 64:(dy + 1) * 64], rhs,
                                start=(dy == 0), stop=(dy == 2),
                                tile_position=(0, 64 * pb))
                    # pool2: tile 0 via Act bias+relu + DVE fp16 max tree,
                    # tile 1 via direct DVE reduce (bias+relu ride the write)
                    t2 = sxp.tile([128, 128], f16, tag="t2")
                    if v == 0:
                        s2 = sxp.tile([128, 512], f16, tag="s2")
                        nc.scalar.activation(s2[:], ps2[:], AF.Relu,
                                             bias=ct['B2R'][:], scale=1.0)
                        th2 = sxp.tile([128, 256], f16, tag="th2")
                        nc.vector.tensor_tensor(
                            th2[:], view(s2[:], 0, 128, 0, [[256, 2], [16, 16], [2, 8]]),
                            view(s2[:], 0, 128, 1, [[256, 2], [16, 16], [2, 8]]),
                            op=ALU.max)
                        nc.vector.tensor_tensor(
                            t2[:], view(th2[:], 0, 128, 0, [[128, 2], [16, 8], [1, 8]]),
                            view(th2[:], 0, 128, 8, [[128, 2], [16, 8], [1, 8]]),
                            op=ALU.max)
                    else:
                        nc.vector.tensor_reduce(
                            out=t2[:],
                            in_=view(ps2[:], 0, 128, 0,
                                     [[256, 2], [32, 8], [2, 8], [16, 2], [1, 2]]),
                            op=ALU.max, axis=AX.XY)
                    for i in range(4):
                        iu = 4 * v + i
                        pb, st = i % 2, i // 2
                        dstv = view(r3[:], 0, 64, iu * 100 + 11, [[10, 8], [1, 8]])
                        srcv = view(t2[:], 64 * pb, 64, st * 64, [[8, 8], [1, 8]])
                        scal = (zp if v == 0 else ct['B2R'])[64 * pb:64 * pb + 64, :]
                        if i % 2 == 0:
                            nc.vector.tensor_scalar(dstv, srcv, scal, 0.0,
                                                    op0=ALU.add, op1=ALU.max)
                        else:
                            nc.gpsimd.tensor_scalar(dstv, srcv, scal, 0.0,
                                                    op0=ALU.add, op1=ALU.max)
                # dx-shift for conv3 (parts 64:128 = planes shifted by 1)
                src = bass.AP(tensor=r3[:].tensor, offset=r3[:].offset + 1,
                              ap=[[r3[:].ap[0][0], 64], [1, 800]])
                nc.sync.dma_start(r3[64:128, 0:800], src)

                # conv3: 6 accumulating passes, full 512-wide stream
                ps3 = ps3p.tile([128, 512], f32, tag="ps3")
                for dy in range(3):
                    rhsA = view(r3[:], 0, 128, dy * 10, [[100, 8], [10, 8], [1, 8]])
                    nc.tensor.matmul(ps3[:], ct['W3P'][:, (2 * dy) * 128:(2 * dy + 1) * 128],
                                     rhsA, start=(dy == 0), stop=False)
                    rhsB = view(r3[:], 0, 64, dy * 10 + 2, [[100, 8], [10, 8], [1, 8]])
                    nc.tensor.matmul(ps3[:], ct['W3P'][0:64, (2 * dy + 1) * 128:(2 * dy + 2) * 128],
                                     rhsB, start=False, stop=(dy == 2))
                # relu+bias then avgpool adds into ef (0.25 folded into W1FC)
                h3 = sxp.tile([128, 512], f16, tag="h3")
                nc.scalar.activation(h3[:], ps3[:], AF.Relu, bias=ct['B3'][:],
                                     scale=1.0)
                with nc.allow_low_precision("avgpool sums 4 values; fp16 is fine"):
                    e1 = sxp.tile([128, 256], f16, tag="e1")
                    nc.vector.tensor_tensor(
                        e1[:], view(h3[:], 0, 128, 0, [[64, 8], [8, 8], [2, 4]]),
                        view(h3[:], 0, 128, 1, [[64, 8], [8, 8], [2, 4]]), op=ALU.add)
                    efd = view(ef[:], 0, 128, g * 128, [[16, 8], [4, 4], [1, 4]])
                    nc.vector.tensor_tensor(
                        efd, view(e1[:], 0, 128, 0, [[32, 8], [8, 4], [1, 4]]),
                        view(e1[:], 0, 128, 4, [[32, 8], [8, 4], [1, 4]]), op=ALU.add)

            for g in range(G):
                stage1(g)
                if g >= 2:
                    stage2(g - 2)
            stage2(G - 2)
            stage2(G - 1)

        # tail-only params load during the last conv groups (Act queue)
        for name in ["W1FC", "B1FC", "W2FC", "B2FC", "URT", "UIT", "WH",
                     "AH", "CH", "H2WT", "H2B"]:
            load_param(name, nc.scalar)

        # ------------------- tail: fc1 / fc2 / quantum / head -------------------
        with tc.tile_pool(name="tsb", bufs=1) as tsb, \
             tc.tile_pool(name="psfp", bufs=1, space="PSUM") as psfp, \
             tc.tile_pool(name="pstp", bufs=2, space="PSUM") as pstp, \
             tc.tile_pool(name="tp1", bufs=2, space="PSUM") as tp1, \
             tc.tile_pool(name="tp2", bufs=2, space="PSUM") as tp2:

            psf = psfp.tile([nb, 512], f32, tag="psf")
            for s in range(16):
                lhsT = view(ef[:], 0, 128, s, [[16, nb]])
                nc.tensor.matmul(psf[:], lhsT,
                                 ct['W1FC'][:, s * 512:(s + 1) * 512],
                                 start=(s == 0), stop=False)
            nc.tensor.matmul(psf[:], ones_r[0:1, 0:nb], ct['B1FC'][:],
                             start=False, stop=True)
            h1t = tsb.tile([nb, 512], f32, tag="h1t")
            nc.scalar.activation(h1t[:], psf[:], AF.Relu)

            h1 = tsb.tile([128, 4 * nb], f32, tag="h1")
            for t in range(4):
                pst = pstp.tile([128, nb], f32, tag="pst")
                nc.tensor.transpose(pst[:], h1t[:, t * 128:(t + 1) * 128], ident[0:nb, 0:nb])
                nc.vector.tensor_copy(h1[:, t * nb:(t + 1) * nb], pst[:])

            psz = tp1.tile([16, nb], f32, tag="tp1")
            for t in range(4):
                nc.tensor.matmul(psz[:], ct['W2FC'][:, t * 16:(t + 1) * 16],
                                 h1[:, t * nb:(t + 1) * nb],
                                 start=(t == 0), stop=(t == 3))
            e = tsb.tile([16, nb], f32, tag="e")
            nc.scalar.activation(e[:], psz[:], AF.Exp, bias=ct['B2FC'][:], scale=1.0)

            psr = tp2.tile([16, nb], f32, tag="tp2")
            nc.tensor.matmul(psr[:], ct['URT'][:], e[:], start=True, stop=True)
            psi2 = tp2.tile([16, nb], f32, tag="tp2")
            nc.tensor.matmul(psi2[:], ct['UIT'][:], e[:], start=True, stop=True)
            tr = tsb.tile([16, nb], f32, tag="tr")
            nc.scalar.square(tr[:], psr[:])
            ti = tsb.tile([16, nb], f32, tag="ti")
            nc.scalar.square(ti[:], psi2[:])
            pun = tsb.tile([16, nb], f32, tag="pun")
            nc.vector.tensor_add(pun[:], tr[:], ti[:])

            pss = tp1.tile([1, nb], f32, tag="tp1")
            nc.tensor.matmul(pss[:], ones_c[:], pun[:], start=True, stop=True)
            rec = tsb.tile([1, nb], f32, tag="rec")
            nc.vector.reciprocal(rec[:], pss[:])
            psb = tp1.tile([16, nb], f32, tag="tp1")
            nc.tensor.matmul(psb[:], ones_r[0:1, 0:16], rec[:], start=True, stop=True)
            bc = tsb.tile([16, nb], f32, tag="bc")
            nc.scalar.copy(bc[:], psb[:])
            pn = tsb.tile([16, nb], f32, tag="pn")
            nc.vector.tensor_mul(pn[:], pun[:], bc[:])

            psy = tp2.tile([128, nb], f32, tag="tp2")
            nc.tensor.matmul(psy[:], ct['WH'][:], pn[:], start=True, stop=True)
            h2 = tsb.tile([128, nb], f32, tag="h2")
            nc.scalar.activation(h2[:], psy[:], AF.Relu, bias=ct['CH'][:], scale=ct['AH'][:])

            pso = tp1.tile([nb, 100], f32, tag="tp1")
            nc.tensor.matmul(pso[:], h2[:], ct['H2WT'][:], start=True, stop=False)
            nc.tensor.matmul(pso[:], ones_r[0:1, 0:nb], ct['H2B'][:],
                             start=False, stop=True)
            outs = tsb.tile([nb, 100], f32, tag="outs")
            nc.scalar.copy(outs[:], pso[:])
            nc.sync.dma_start(OUT[:], outs[:])

    nc.finalize()
    return nc


def get_program(nb=NB):
    key = ("prog", nb)
    if key not in _cache:
        _cache[key] = _build_program(nb)
    return _cache[key]


# ---------------------------------------------------------------------------
# entry point
# ---------------------------------------------------------------------------

def kernel(**inputs):
    from concourse.bass_utils import run_bass_kernel_spmd

    x = np.asarray(inputs['x'], np.float32)
    B = x.shape[0]
    nb = B // NCORES
    hw = _host_weights({k: np.asarray(v) for k, v in inputs.items()})

    nc = get_program(nb)
    in_maps = []
    for c in range(NCORES):
        m = {'xr': _build_xr(x[c * nb:(c + 1) * nb])}
        m.update(hw)
        in_maps.append(m)
    res = run_bass_kernel_spmd(nc, in_maps, core_ids=list(range(NCORES)))
    return np.concatenate([res.results[c]['out'] for c in range(NCORES)], axis=0)


# revision 34
# speedup vs baseline: 1.3136x; 1.0676x over previous
"""Trainium2 Bass kernel for nn_EnhancedHybridModel.

Pipeline per core (pure data parallel over batch, 128 images/core):
  conv1(3->32,3x3,p1)+BN+ReLU -> maxpool2 -> conv2(32->64)+BN+ReLU -> maxpool2
  -> conv3(64->128)+BN+ReLU -> avgpool2 -> fc 2048->512 -> fc 512->16
  -> softmax -> 4-qubit statevector sim (collapses to two fixed real 16x16
  matmuls built on host from q_weights) -> head 4->128->100.

Layout decisions (matmul cost scales with out free size, so every matmul
uses as many of the 128 output partitions as the layer allows):
  * conv1: 4 images block-diagonally packed on the PE (K = 4x(27+bias row),
    M = 4img x 32ch), two F=512 matmuls per 4-image unit.
  * maxpool: horizontal pairs via scalar_tensor_tensor max straight out of
    PSUM (Pool/DVE), vertical pairs via fp16 tensor_tensor max (DVE 2x mode),
    then per-image tensor_scalar writes (DVE 4x / Pool ptr-form) into the
    next conv's padded-plane tile; conv1's relu rides the first max
    (max(a,0,b)), conv2's bias+relu ride the write stage.
  * conv2: 3 dy passes, K=96 dx-preshifted planes (one shift DMA per group).
  * conv3: 6 passes (3dy x {2dx-packed 128, dx2 64}); relu+bias via one Act
    pass, avgpool via fp16 adds with the 0.25 folded into fc1 weights.
  * 2-stage software pipeline (conv1(g) runs two groups ahead of
    conv2/conv3(g)) keeps the PE dense; a dummy-matmul warmup covers the
    first input DMA so the PE p-state ramp completes early.
"""

import numpy as np

NB = 128          # images per core
NCORES = 8
ROWL = 1160       # padded im2col plane length per image (1156 + slack)
EPS = 1e-5

_cache = {}


# ---------------------------------------------------------------------------
# host-side math (quantum layer constants, weight folding, im2col planes)
# ---------------------------------------------------------------------------

def _cnot_ring_matrix():
    M = np.zeros((16, 16), dtype=np.complex64)
    for b in range(16):
        bb = b
        for cw, tw in [(0, 1), (1, 2), (2, 3), (3, 0)]:
            if (bb >> (3 - cw)) & 1:
                bb ^= 1 << (3 - tw)
        M[bb, b] = 1.0
    return M


def _zsigns():
    return np.array([[1.0 - 2.0 * ((b >> (3 - w)) & 1) for b in range(16)]
                     for w in range(4)], dtype=np.float32)


def _quantum_unitary(q_weights):
    CN = _cnot_ring_matrix()
    U_tot = np.eye(16, dtype=np.complex64)
    for l in range(2):
        c = np.cos(q_weights[l] * 0.5).astype(np.complex64)
        s = np.sin(q_weights[l] * 0.5).astype(np.complex64)
        U = np.ones((1, 1), dtype=np.complex64)
        for q in range(4):
            g = np.array([[c[q], -1j * s[q]], [-1j * s[q], c[q]]], dtype=np.complex64)
            U = np.kron(U, g)
        U_tot = (CN @ U) @ U_tot
    return U_tot  # psi_out = psi_in @ U_tot.T


def _host_weights(inp):
    f32, f16 = np.float32, np.float16
    sc = f32(1.0 / np.sqrt(1.0 + EPS))
    out = {}

    # conv1: 4-image block-diagonal [128, 128]; row 32j+27 carries the bias.
    g1 = inp['bn1_g'] * sc
    w1 = np.zeros((28, 32), f32)
    for dy in range(3):
        for dx in range(3):
            for ci in range(3):
                w1[(dy * 3 + dx) * 3 + ci, :] = inp['conv1_w'][:, ci, dy, dx] * g1
    w1[27, :] = inp['conv1_b'] * g1 + inp['bn1_b']
    w1blk = np.zeros((128, 128), f16)
    for j in range(4):
        w1blk[32 * j:32 * j + 28, 32 * j:32 * j + 32] = w1
    out['W1BLK'] = w1blk

    # conv2: [96, 3*64] rows dx*32+ci; bias applied at the pool write stage.
    g2 = inp['bn2_g'] * sc
    w2 = np.zeros((96, 192), f32)
    for dy in range(3):
        for dx in range(3):
            for ci in range(32):
                w2[dx * 32 + ci, dy * 64:(dy + 1) * 64] = inp['conv2_w'][:, ci, dy, dx] * g2
    out['W2DY'] = w2.astype(f16)
    out['B2R'] = np.tile(inp['conv2_b'] * g2 + inp['bn2_b'], 2)[:, None].astype(f32)

    # conv3: 6 passes [128, 6*128]: pass 2*dy   = chunkA (dx0,dx1 on 128 rows)
    #                               pass 2*dy+1 = chunkB (dx2 on 64 rows)
    g3 = inp['bn3_g'] * sc
    w3 = np.zeros((128, 6 * 128), f32)
    for dy in range(3):
        for ci in range(64):
            for dx in range(2):
                w3[dx * 64 + ci, (2 * dy) * 128:(2 * dy + 1) * 128] = \
                    inp['conv3_w'][:, ci, dy, dx] * g3
            w3[ci, (2 * dy + 1) * 128:(2 * dy + 2) * 128] = \
                inp['conv3_w'][:, ci, dy, 2] * g3
    out['W3P'] = w3.astype(f16)
    out['B3'] = (inp['conv3_b'] * g3 + inp['bn3_b']).astype(f32)[:, None]

    # fc1 with avgpool folded in: input index = c*16 + s, pool = 0.25*sum
    fr1 = inp['fr1_w'].reshape(512, 128, 16)  # [m, c, s]
    w1fc = np.zeros((128, 16 * 512), f32)
    for s in range(16):
        w1fc[:, s * 512:(s + 1) * 512] = (fr1[:, :, s].T * 0.25)
    out['W1FC'] = w1fc.astype(f16)
    out['B1FC'] = inp['fr1_b'].astype(f32)[None, :]

    fr2 = inp['fr2_w']  # [16, 512]
    w2fc = np.zeros((128, 64), f32)
    for t in range(4):
        w2fc[:, t * 16:(t + 1) * 16] = fr2[:, t * 128:(t + 1) * 128].T
    out['W2FC'] = w2fc
    out['B2FC'] = inp['fr2_b'].astype(f32)[:, None]

    U = _quantum_unitary(np.asarray(inp['q_weights'], np.float64))
    # stacked [Re(U) | Im(U)] so one matmul produces both statevector parts
    urti = np.zeros((16, 48), f32)
    urti[:, 0:16] = np.real(U).T
    urti[:, 32:48] = np.imag(U).T   # at col 32 so Act reads a 32-aligned slice
    out['URTI'] = urti

    ZS = _zsigns()
    out['WH'] = np.ascontiguousarray((inp['h1_w'] @ ZS).T.astype(f32))  # [16, 128]
    ah = inp['bnh_g'] * sc
    out['AH'] = ah.astype(f32)[:, None]
    out['CH'] = (ah * inp['h1_b'] + inp['bnh_b']).astype(f32)[:, None]

    out['H2WT'] = np.ascontiguousarray(inp['h2_w'].T.astype(f32))  # [128, 100]
    out['H2B'] = inp['h2_b'].astype(f32)[None, :]
    return out


def _build_xr(x):
    """Host im2col planes for conv1, packed [128, (B//4)*ROWL]: partition
    32*j + r holds plane row r of image 4u+j at columns [u*ROWL, ...): the
    27 shifted flat padded planes, a ones row (r=27) for the bias, zeros
    above."""
    B = x.shape[0]
    U = B // 4
    xp = np.zeros((B, 3, 34 * 34 + 72), np.float16)  # flat plane + shift slack
    xpv = xp[:, :, :34 * 34].reshape(B, 3, 34, 34)
    xpv[:, :, 1:33, 1:33] = x
    xr = np.zeros((32, B, ROWL), np.float16)
    for dy in range(3):
        for dx in range(3):
            sh = dy * 34 + dx
            for ci in range(3):
                r = (dy * 3 + dx) * 3 + ci
                xr[r, :, :1156] = xp[:, ci, sh:sh + 1156]
    xr[27, :, :] = 1.0
    # [32, B, ROWL] -> [32, U, 4, ROWL] -> [4, 32, U, ROWL] -> [128, U*ROWL]
    xrb = xr.reshape(32, U, 4, ROWL).transpose(2, 0, 1, 3)
    return np.ascontiguousarray(xrb.reshape(128, U * ROWL))


# ---------------------------------------------------------------------------
# device program
# ---------------------------------------------------------------------------

def _build_program(nb):
    import concourse.bass as bass
    import concourse.tile as tile
    from concourse import bacc, mybir
    from concourse.masks import make_identity
    from contextlib import ExitStack

    f32 = mybir.dt.float32
    f16 = mybir.dt.float16
    AF = mybir.ActivationFunctionType
    ALU = mybir.AluOpType
    AX = mybir.AxisListType

    def view(base_ap, part_start, nparts, free_off, free_dims):
        pitch = base_ap.ap[0][0]
        return bass.AP(tensor=base_ap.tensor,
                       offset=base_ap.offset + part_start * pitch + free_off,
                       ap=[[pitch, nparts]] + [list(d) for d in free_dims])

    nc = bacc.Bacc("TRN2", target_bir_lowering=False)
    G = nb // 8            # groups of 8 images
    NU = nb // 4           # units of 4 images

    XR = nc.declare_dram_parameter("xr", [128, NU * ROWL], f16, isOutput=False)
    dparams = {}
    for name, shape, dt in [("W1BLK", [128, 128], f16), ("W2DY", [96, 192], f16),
                            ("B2R", [128, 1], f32),
                            ("W3P", [128, 768], f16), ("B3", [128, 1], f32),
                            ("W1FC", [128, 16 * 512], f16), ("B1FC", [1, 512], f32),
                            ("W2FC", [128, 64], f32), ("B2FC", [16, 1], f32),
                            ("URTI", [16, 48], f32),
                            ("WH", [16, 128], f32), ("AH", [128, 1], f32),
                            ("CH", [128, 1], f32),
                            ("H2WT", [128, 100], f32), ("H2B", [1, 100], f32)]:
        dparams[name] = nc.declare_dram_parameter(name, shape, dt, isOutput=False)
    OUT = nc.declare_dram_parameter("out", [nb, 100], f32, isOutput=True)

    R2N = 8 * 324 + 16     # 8 image planes of 18x18 + slack
    R3N = 8 * 100 + 8

    with tile.TileContext(nc) as tc, ExitStack() as ctx:
        const = ctx.enter_context(tc.tile_pool(name="const", bufs=1))
        ct = {}

        def load_param(name, eng):
            t = const.tile(list(dparams[name].shape), dparams[name].dtype,
                           tag="c_" + name)
            eng.dma_start(t[:], dparams[name][:])
            ct[name] = t

        r1p = ctx.enter_context(tc.tile_pool(name="r1p", bufs=6))
        r1t = [None] * G

        def load_r1(g):
            # two per-unit DMAs so conv1 on unit 0 starts before unit 1 lands
            r1 = r1p.tile([128, 2 * ROWL], f16, tag="r1")
            for u in range(2):
                nc.sync.dma_start(r1[:, u * ROWL:(u + 1) * ROWL],
                                  XR[:, (g * 2 + u) * ROWL:(g * 2 + u + 1) * ROWL])
            r1t[g] = r1

        # critical-path DMAs on SP, in consumption order
        load_param("W1BLK", nc.sync)
        load_r1(0)
        load_r1(1)
        load_param("W2DY", nc.sync)
        load_param("B2R", nc.sync)
        load_r1(2)
        load_param("W3P", nc.sync)
        load_param("B3", nc.sync)
        load_r1(3)

        ident = const.tile([128, 128], f32, tag="ident")
        make_identity(nc, ident[:])
        ones_r = const.tile([1, 128], f32, tag="ones_r")
        nc.vector.memset(ones_r[:], 1.0)
        ones_c = const.tile([16, 1], f32, tag="ones_c")
        nc.vector.memset(ones_c[:], 1.0)
        zp = const.tile([128, 1], f32, tag="zp")
        nc.vector.memset(zp[:], 0.0)
        ef = const.tile([128, nb * 16], f16, tag="ef")   # fc1 input accumulator

        # --- persistent triple-buffered conv input tiles with zero borders ---
        r2b, r3b = [], []
        for k in range(5):
            r2 = const.tile([128, R2N], f16, tag=f"r2_{k}")
            nc.gpsimd.memset(view(r2[:], 0, 32, 0, [[324, 8], [17 * 18, 2], [1, 18]]), 0.0)
            nc.gpsimd.memset(view(r2[:], 0, 32, 0, [[324, 8], [18, 18], [17, 2]]), 0.0)
            nc.gpsimd.memset(r2[0:32, 8 * 324:R2N], 0.0)
            r2b.append(r2)
            r3 = const.tile([128, R3N], f16, tag=f"r3_{k}")
            nc.gpsimd.memset(view(r3[:], 0, 64, 0, [[100, 8], [9 * 10, 2], [1, 10]]), 0.0)
            nc.gpsimd.memset(view(r3[:], 0, 64, 0, [[100, 8], [10, 10], [9, 2]]), 0.0)
            nc.gpsimd.memset(r3[0:64, 800:R3N], 0.0)
            r3b.append(r3)

        # --- PE p-state warmup: dummy matmuls while the first DMAs land ---
        with tc.tile_pool(name="wup", bufs=1) as wup, \
             tc.tile_pool(name="wps", bufs=1, space="PSUM") as wps:
            w0 = wup.tile([128, 512], f16, tag="w0")
            nc.vector.memset(w0[:], 0.0)
            pw = wps.tile([128, 512], f32, tag="pw")
            for _ in range(8):
                nc.tensor.matmul(pw[:], w0[:, 0:128], w0[:], start=True, stop=True)

        with tc.tile_pool(name="sxp", bufs=3) as sxp, \
             tc.tile_pool(name="ps1p", bufs=2, space="PSUM") as ps1p, \
             tc.tile_pool(name="ps2p", bufs=2, space="PSUM") as ps2p, \
             tc.tile_pool(name="ps3p", bufs=2, space="PSUM") as ps3p:

            def stage1(g):
                """conv1 + pool1 for group g (8 images, 2 units)."""
                if g + 4 < G:
                    load_r1(g + 4)
                r1 = r1t[g]
                r2 = r2b[g % 5]
                for u in range(2):
                    ps1 = ps1p.tile([128, 1024], f32, tag="ps1")
                    for h in range(2):
                        rhs = view(r1[:], 0, 128, u * ROWL + h * 16 * 34,
                                   [[34, 16], [1, 32]])
                        nc.tensor.matmul(ps1[:, h * 512:(h + 1) * 512],
                                         ct['W1BLK'][:], rhs, start=True, stop=True)
                    # pool1: unit 0 evacuates via Act relu + DVE fp16 max tree,
                    # unit 1 via a direct DVE 4:1 max reduce (relu rides the
                    # per-image write's max-with-0); both end in t1 [128, 256]
                    t1 = sxp.tile([128, 256], f16, tag="t1")
                    if u == 0:
                        s1 = sxp.tile([128, 1024], f16, tag="s1")
                        nc.scalar.activation(s1[:], ps1[:], AF.Relu)
                        th = sxp.tile([128, 512], f16, tag="th")
                        nc.vector.tensor_tensor(
                            th[:], view(s1[:], 0, 128, 0, [[32, 32], [2, 16]]),
                            view(s1[:], 0, 128, 1, [[32, 32], [2, 16]]), op=ALU.max)
                        nc.vector.tensor_tensor(
                            t1[:], view(th[:], 0, 128, 0, [[32, 16], [1, 16]]),
                            view(th[:], 0, 128, 16, [[32, 16], [1, 16]]), op=ALU.max)
                    else:
                        nc.vector.tensor_reduce(
                            out=t1[:],
                            in_=view(ps1[:], 0, 128, 0,
                                     [[64, 16], [2, 16], [32, 2], [1, 2]]),
                            op=ALU.max, axis=AX.XY)
                    for j in range(4):
                        iu = 4 * u + j
                        dstv = view(r2[:], 0, 32, iu * 324 + 19, [[18, 16], [1, 16]])
                        srcv = view(t1[:], 32 * j, 32, 0, [[16, 16], [1, 16]])
                        if j % 2 == 0:
                            nc.vector.tensor_scalar(dstv, srcv, zp[32 * j:32 * j + 32, :],
                                                    0.0, op0=ALU.add, op1=ALU.max)
                        else:
                            nc.gpsimd.tensor_scalar(dstv, srcv, zp[32 * j:32 * j + 32, :],
                                                    0.0, op0=ALU.add, op1=ALU.max)
                # dx-shift copies for conv2's K-packing (parts 32:96)
                pitch = r2[:].ap[0][0]
                for dx in (1, 2):
                    src = bass.AP(tensor=r2[:].tensor, offset=r2[:].offset + dx,
                                  ap=[[pitch, 32], [1, 8 * 324 + 4 - dx]])
                    eng = nc.sync if dx == 1 else nc.scalar
                    eng.dma_start(r2[32 * dx:32 * dx + 32, 0:8 * 324 + 4 - dx],
                                  src)

            def stage2(g):
                """conv2 + pool2 + conv3 + avgpool for group g."""
                r2 = r2b[g % 5]
                r3 = r3b[g % 5]
                for v in range(2):
                    ps2 = ps2p.tile([128, 512], f32, tag="ps2")
                    for i in range(4):            # image 4v+i within group
                        iu = 4 * v + i
                        pb, st = i % 2, i // 2    # partition block, subtile
                        for dy in range(3):
                            rhs = view(r2[:], 0, 96, iu * 324 + dy * 18,
                                       [[18, 16], [1, 16]])
                            nc.tensor.matmul(
                                ps2[64 * pb:64 * pb + 64, st * 256:st * 256 + 256],
                                ct['W2DY'][:, dy * 64:(dy + 1) * 64], rhs,
                                start=(dy == 0), stop=(dy == 2),
                                tile_position=(0, 64 * pb))
                    # pool2: tile 0 via Act bias+relu + DVE fp16 max tree,
                    # tile 1 via direct DVE reduce (bias+relu ride the write)
                    t2 = sxp.tile([128, 128], f16, tag="t2")
                    if v == 0:
                        s2 = sxp.tile([128, 512], f16, tag="s2")
                        nc.scalar.activation(s2[:], ps2[:], AF.Relu,
                                             bias=ct['B2R'][:], scale=1.0)
                        th2 = sxp.tile([128, 256], f16, tag="th2")
                        nc.vector.tensor_tensor(
                            th2[:], view(s2[:], 0, 128, 0, [[256, 2], [16, 16], [2, 8]]),
                            view(s2[:], 0, 128, 1, [[256, 2], [16, 16], [2, 8]]),
                            op=ALU.max)
                        nc.vector.tensor_tensor(
                            t2[:], view(th2[:], 0, 128, 0, [[128, 2], [16, 8], [1, 8]]),
                            view(th2[:], 0, 128, 8, [[128, 2], [16, 8], [1, 8]]),
                            op=ALU.max)
                    else:
                        nc.vector.tensor_reduce(
                            out=t2[:],
                            in_=view(ps2[:], 0, 128, 0,
                                     [[256, 2], [32, 8], [2, 8], [16, 2], [1, 2]]),
                            op=ALU.max, axis=AX.XY)
                    for i in range(4):
                        iu = 4 * v + i
                        pb, st = i % 2, i // 2
                        dstv = view(r3[:], 0, 64, iu * 100 + 11, [[10, 8], [1, 8]])
                        srcv = view(t2[:], 64 * pb, 64, st * 64, [[8, 8], [1, 8]])
                        scal = (zp if v == 0 else ct['B2R'])[64 * pb:64 * pb + 64, :]
                        if i % 2 == 0:
                            nc.vector.tensor_scalar(dstv, srcv, scal, 0.0,
                                                    op0=ALU.add, op1=ALU.max)
                        else:
                            nc.gpsimd.tensor_scalar(dstv, srcv, scal, 0.0,
                                                    op0=ALU.add, op1=ALU.max)
                # dx-shift for conv3 (parts 64:128 = planes shifted by 1)
                src = bass.AP(tensor=r3[:].tensor, offset=r3[:].offset + 1,
                              ap=[[r3[:].ap[0][0], 64], [1, 800]])
                nc.gpsimd.dma_start(r3[64:128, 0:800], src)

                # conv3: 6 accumulating passes, full 512-wide stream
                ps3 = ps3p.tile([128, 512], f32, tag="ps3")
                for dy in range(3):
                    rhsA = view(r3[:], 0, 128, dy * 10, [[100, 8], [10, 8], [1, 8]])
                    nc.tensor.matmul(ps3[:], ct['W3P'][:, (2 * dy) * 128:(2 * dy + 1) * 128],
                                     rhsA, start=(dy == 0), stop=False)
                    rhsB = view(r3[:], 0, 64, dy * 10 + 2, [[100, 8], [10, 8], [1, 8]])
                    nc.tensor.matmul(ps3[:], ct['W3P'][0:64, (2 * dy + 1) * 128:(2 * dy + 2) * 128],
                                     rhsB, start=False, stop=(dy == 2))
                # relu+bias then avgpool adds into ef (0.25 folded into W1FC)
                h3 = sxp.tile([128, 512], f16, tag="h3")
                nc.scalar.activation(h3[:], ps3[:], AF.Relu, bias=ct['B3'][:],
                                     scale=1.0)
                with nc.allow_low_precision("avgpool sums 4 values; fp16 is fine"):
                    e1 = sxp.tile([128, 256], f16, tag="e1")
                    nc.vector.tensor_tensor(
                        e1[:], view(h3[:], 0, 128, 0, [[64, 8], [8, 8], [2, 4]]),
                        view(h3[:], 0, 128, 1, [[64, 8], [8, 8], [2, 4]]), op=ALU.add)
                    efd = view(ef[:], 0, 128, g * 128, [[16, 8], [4, 4], [1, 4]])
                    nc.vector.tensor_tensor(
                        efd, view(e1[:], 0, 128, 0, [[32, 8], [8, 4], [1, 4]]),
                        view(e1[:], 0, 128, 4, [[32, 8], [8, 4], [1, 4]]), op=ALU.add)

            for g in range(G):
                stage1(g)
                if g >= 4:
                    stage2(g - 4)
            for g in range(G - 4, G):
                stage2(g)

        # tail-only params: big W1FC via gpsimd SWDGE (cheap engine-side),
        # the small ones on Act
        load_param("W1FC", nc.gpsimd)
        for name in ["B1FC", "W2FC", "B2FC", "URTI", "WH",
                     "AH", "CH", "H2WT", "H2B"]:
            load_param(name, nc.scalar)

        # ------------------- tail: fc1 / fc2 / quantum / head -------------------
        with tc.tile_pool(name="tsb", bufs=1) as tsb, \
             tc.tile_pool(name="psfp", bufs=1, space="PSUM") as psfp, \
             tc.tile_pool(name="pstp", bufs=2, space="PSUM") as pstp, \
             tc.tile_pool(name="tp1", bufs=2, space="PSUM") as tp1, \
             tc.tile_pool(name="tp2", bufs=2, space="PSUM") as tp2:

            psf = psfp.tile([nb, 512], f32, tag="psf")
            for s in range(16):
                lhsT = view(ef[:], 0, 128, s, [[16, nb]])
                nc.tensor.matmul(psf[:], lhsT,
                                 ct['W1FC'][:, s * 512:(s + 1) * 512],
                                 start=(s == 0), stop=False)
            nc.tensor.matmul(psf[:], ones_r[0:1, 0:nb], ct['B1FC'][:],
                             start=False, stop=True)
            h1t = tsb.tile([nb, 512], f32, tag="h1t")
            nc.scalar.activation(h1t[:], psf[:], AF.Relu)

            h1 = tsb.tile([128, 4 * nb], f32, tag="h1")
            for t in range(4):
                pst = pstp.tile([128, nb], f32, tag="pst")
                nc.tensor.transpose(pst[:], h1t[:, t * 128:(t + 1) * 128], ident[0:nb, 0:nb])
                nc.vector.tensor_copy(h1[:, t * nb:(t + 1) * nb], pst[:])

            psz = tp1.tile([16, nb], f32, tag="tp1")
            for t in range(4):
                nc.tensor.matmul(psz[:], ct['W2FC'][:, t * 16:(t + 1) * 16],
                                 h1[:, t * nb:(t + 1) * nb],
                                 start=(t == 0), stop=(t == 3))
            e = tsb.tile([16, nb], f32, tag="e")
            nc.scalar.activation(e[:], psz[:], AF.Exp, bias=ct['B2FC'][:], scale=1.0)

            # two parallel branches off e:
            #   A: |U e|^2 (statevector, unnormalized), B: 1/||e||^2 broadcast
            # (U unitary => ||U e|| = ||e||, so B comes straight from e)
            psru = tp2.tile([48, nb], f32, tag="tp2")
            nc.tensor.matmul(psru[:], ct['URTI'][:], e[:], start=True, stop=True)
            e2 = tsb.tile([16, nb], f32, tag="e2")
            nc.vector.tensor_mul(e2[:], e[:], e[:])
            pss = tp1.tile([1, nb], f32, tag="tp1")
            nc.tensor.matmul(pss[:], ones_c[:], e2[:], start=True, stop=True)
            tr = tsb.tile([16, nb], f32, tag="tr")
            nc.scalar.square(tr[:], psru[0:16, :])
            ti = tsb.tile([16, nb], f32, tag="ti")
            nc.scalar.square(ti[:], psru[32:48, :])
            rec = tsb.tile([1, nb], f32, tag="rec")
            nc.vector.reciprocal(rec[:], pss[:])
            pun = tsb.tile([16, nb], f32, tag="pun")
            nc.vector.tensor_add(pun[:], tr[:], ti[:])
            psq = tp2.tile([128, nb], f32, tag="tp2")
            nc.tensor.matmul(psq[:], ct['WH'][:], pun[:], start=True, stop=True)
            psb = tp1.tile([128, nb], f32, tag="tp1")
            nc.tensor.matmul(psb[:], ones_r[0:1, 0:128], rec[:], start=True, stop=True)
            bcs = tsb.tile([128, nb], f32, tag="bcs")
            nc.vector.tensor_copy(bcs[:], psb[:])
            hm2 = tsb.tile([128, nb], f32, tag="hm2")
            nc.vector.tensor_mul(hm2[:], psq[:], bcs[:])
            h2 = tsb.tile([128, nb], f32, tag="h2")
            nc.scalar.activation(h2[:], hm2[:], AF.Relu, bias=ct['CH'][:], scale=ct['AH'][:])

            pso = tp1.tile([nb, 100], f32, tag="tp1")
            nc.tensor.matmul(pso[:], h2[:], ct['H2WT'][:], start=True, stop=False)
            nc.tensor.matmul(pso[:], ones_r[0:1, 0:nb], ct['H2B'][:],
                             start=False, stop=True)
            outs = tsb.tile([nb, 100], f32, tag="outs")
            nc.scalar.copy(outs[:], pso[:])
            nc.sync.dma_start(OUT[:], outs[:])

    nc.finalize()
    return nc


def get_program(nb=NB):
    key = ("prog", nb)
    if key not in _cache:
        _cache[key] = _build_program(nb)
    return _cache[key]


# ---------------------------------------------------------------------------
# entry point
# ---------------------------------------------------------------------------

def kernel(**inputs):
    from concourse.bass_utils import run_bass_kernel_spmd

    x = np.asarray(inputs['x'], np.float32)
    B = x.shape[0]
    nb = B // NCORES
    hw = _host_weights({k: np.asarray(v) for k, v in inputs.items()})

    nc = get_program(nb)
    in_maps = []
    for c in range(NCORES):
        m = {'xr': _build_xr(x[c * nb:(c + 1) * nb])}
        m.update(hw)
        in_maps.append(m)
    res = run_bass_kernel_spmd(nc, in_maps, core_ids=list(range(NCORES)))
    return np.concatenate([res.results[c]['out'] for c in range(NCORES)], axis=0)


# revision 42
# speedup vs baseline: 1.3236x; 1.0076x over previous
"""Trainium2 Bass kernel for nn_EnhancedHybridModel.

Pipeline per core (pure data parallel over batch, 128 images/core):
  conv1(3->32,3x3,p1)+BN+ReLU -> maxpool2 -> conv2(32->64)+BN+ReLU -> maxpool2
  -> conv3(64->128)+BN+ReLU -> avgpool2 -> fc 2048->512 -> fc 512->16
  -> softmax -> 4-qubit statevector sim (collapses to two fixed real 16x16
  matmuls built on host from q_weights) -> head 4->128->100.

Layout decisions (matmul cost scales with out free size, so every matmul
uses as many of the 128 output partitions as the layer allows):
  * conv1: 4 images block-diagonally packed on the PE (K = 4x(27+bias row),
    M = 4img x 32ch), two F=512 matmuls per 4-image unit.
  * maxpool: horizontal pairs via scalar_tensor_tensor max straight out of
    PSUM (Pool/DVE), vertical pairs via fp16 tensor_tensor max (DVE 2x mode),
    then per-image tensor_scalar writes (DVE 4x / Pool ptr-form) into the
    next conv's padded-plane tile; conv1's relu rides the first max
    (max(a,0,b)), conv2's bias+relu ride the write stage.
  * conv2: 3 dy passes, K=96 dx-preshifted planes (one shift DMA per group).
  * conv3: 6 passes (3dy x {2dx-packed 128, dx2 64}); relu+bias via one Act
    pass, avgpool via fp16 adds with the 0.25 folded into fc1 weights.
  * 2-stage software pipeline (conv1(g) runs two groups ahead of
    conv2/conv3(g)) keeps the PE dense; a dummy-matmul warmup covers the
    first input DMA so the PE p-state ramp completes early.
"""

import numpy as np

NB = 128          # images per core
NCORES = 8
ROWL = 1160       # padded im2col plane length per image (1156 + slack)
EPS = 1e-5

_cache = {}


# ---------------------------------------------------------------------------
# host-side math (quantum layer constants, weight folding, im2col planes)
# ---------------------------------------------------------------------------

def _cnot_ring_matrix():
    M = np.zeros((16, 16), dtype=np.complex64)
    for b in range(16):
        bb = b
        for cw, tw in [(0, 1), (1, 2), (2, 3), (3, 0)]:
            if (bb >> (3 - cw)) & 1:
                bb ^= 1 << (3 - tw)
        M[bb, b] = 1.0
    return M


def _zsigns():
    return np.array([[1.0 - 2.0 * ((b >> (3 - w)) & 1) for b in range(16)]
                     for w in range(4)], dtype=np.float32)


def _quantum_unitary(q_weights):
    CN = _cnot_ring_matrix()
    U_tot = np.eye(16, dtype=np.complex64)
    for l in range(2):
        c = np.cos(q_weights[l] * 0.5).astype(np.complex64)
        s = np.sin(q_weights[l] * 0.5).astype(np.complex64)
        U = np.ones((1, 1), dtype=np.complex64)
        for q in range(4):
            g = np.array([[c[q], -1j * s[q]], [-1j * s[q], c[q]]], dtype=np.complex64)
            U = np.kron(U, g)
        U_tot = (CN @ U) @ U_tot
    return U_tot  # psi_out = psi_in @ U_tot.T


def _host_weights(inp):
    f32, f16 = np.float32, np.float16
    sc = f32(1.0 / np.sqrt(1.0 + EPS))
    out = {}

    # conv1: 4-image block-diagonal [128, 128]; row 32j+27 carries the bias.
    g1 = inp['bn1_g'] * sc
    w1 = np.zeros((28, 32), f32)
    for dy in range(3):
        for dx in range(3):
            for ci in range(3):
                w1[(dy * 3 + dx) * 3 + ci, :] = inp['conv1_w'][:, ci, dy, dx] * g1
    w1[27, :] = inp['conv1_b'] * g1 + inp['bn1_b']
    w1blk = np.zeros((128, 128), f16)
    for j in range(4):
        w1blk[32 * j:32 * j + 28, 32 * j:32 * j + 32] = w1
    out['W1BLK'] = w1blk

    # conv2: [96, 3*64] rows dx*32+ci; bias applied at the pool write stage.
    g2 = inp['bn2_g'] * sc
    w2 = np.zeros((96, 192), f32)
    for dy in range(3):
        for dx in range(3):
            for ci in range(32):
                w2[dx * 32 + ci, dy * 64:(dy + 1) * 64] = inp['conv2_w'][:, ci, dy, dx] * g2
    out['W2DY'] = w2.astype(f16)
    out['B2R'] = np.tile(inp['conv2_b'] * g2 + inp['bn2_b'], 2)[:, None].astype(f32)

    # conv3: 6 passes [128, 6*128]: pass 2*dy   = chunkA (dx0,dx1 on 128 rows)
    #                               pass 2*dy+1 = chunkB (dx2 on 64 rows)
    g3 = inp['bn3_g'] * sc
    w3 = np.zeros((128, 6 * 128), f32)
    for dy in range(3):
        for ci in range(64):
            for dx in range(2):
                w3[dx * 64 + ci, (2 * dy) * 128:(2 * dy + 1) * 128] = \
                    inp['conv3_w'][:, ci, dy, dx] * g3
            w3[ci, (2 * dy + 1) * 128:(2 * dy + 2) * 128] = \
                inp['conv3_w'][:, ci, dy, 2] * g3
    out['W3P'] = w3.astype(f16)
    out['B3'] = (inp['conv3_b'] * g3 + inp['bn3_b']).astype(f32)[:, None]

    # fc1 with avgpool folded in: input index = c*16 + s, pool = 0.25*sum
    fr1 = inp['fr1_w'].reshape(512, 128, 16)  # [m, c, s]
    w1fc = np.zeros((128, 16 * 512), f32)
    for s in range(16):
        w1fc[:, s * 512:(s + 1) * 512] = (fr1[:, :, s].T * 0.25)
    out['W1FC'] = w1fc.astype(f16)
    out['B1FC'] = inp['fr1_b'].astype(f32)[None, :]

    fr2 = inp['fr2_w']  # [16, 512]
    w2fc = np.zeros((128, 64), f32)
    for t in range(4):
        w2fc[:, t * 16:(t + 1) * 16] = fr2[:, t * 128:(t + 1) * 128].T
    out['W2FC'] = w2fc
    out['B2FC'] = inp['fr2_b'].astype(f32)[:, None]

    U = _quantum_unitary(np.asarray(inp['q_weights'], np.float64))
    # stacked [Re(U) | Im(U)] so one matmul produces both statevector parts
    urti = np.zeros((16, 48), f32)
    urti[:, 0:16] = np.real(U).T
    urti[:, 32:48] = np.imag(U).T   # at col 32 so Act reads a 32-aligned slice
    out['URTI'] = urti

    ZS = _zsigns()
    out['WH'] = np.ascontiguousarray((inp['h1_w'] @ ZS).T.astype(f32))  # [16, 128]
    ah = inp['bnh_g'] * sc
    out['AH'] = ah.astype(f32)[:, None]
    out['CH'] = (ah * inp['h1_b'] + inp['bnh_b']).astype(f32)[:, None]

    out['H2WT'] = np.ascontiguousarray(inp['h2_w'].T.astype(f32))  # [128, 100]
    out['H2B'] = inp['h2_b'].astype(f32)[None, :]
    return out


def _build_xr(x):
    """Host im2col planes for conv1, packed [128, (B//4)*ROWL]: partition
    32*j + r holds plane row r of image 4u+j at columns [u*ROWL, ...): the
    27 shifted flat padded planes, a ones row (r=27) for the bias, zeros
    above."""
    B = x.shape[0]
    U = B // 4
    xp = np.zeros((B, 3, 34 * 34 + 72), np.float16)  # flat plane + shift slack
    xpv = xp[:, :, :34 * 34].reshape(B, 3, 34, 34)
    xpv[:, :, 1:33, 1:33] = x
    xr = np.zeros((32, B, ROWL), np.float16)
    for dy in range(3):
        for dx in range(3):
            sh = dy * 34 + dx
            for ci in range(3):
                r = (dy * 3 + dx) * 3 + ci
                xr[r, :, :1156] = xp[:, ci, sh:sh + 1156]
    xr[27, :, :] = 1.0
    # [32, B, ROWL] -> [32, U, 4, ROWL] -> [4, 32, U, ROWL] -> [128, U*ROWL]
    xrb = xr.reshape(32, U, 4, ROWL).transpose(2, 0, 1, 3)
    return np.ascontiguousarray(xrb.reshape(128, U * ROWL))


# ---------------------------------------------------------------------------
# device program
# ---------------------------------------------------------------------------

def _build_program(nb):
    import concourse.bass as bass
    import concourse.tile as tile
    from concourse import bacc, mybir
    from concourse.masks import make_identity
    from contextlib import ExitStack

    f32 = mybir.dt.float32
    f16 = mybir.dt.float16
    AF = mybir.ActivationFunctionType
    ALU = mybir.AluOpType
    AX = mybir.AxisListType

    def view(base_ap, part_start, nparts, free_off, free_dims):
        pitch = base_ap.ap[0][0]
        return bass.AP(tensor=base_ap.tensor,
                       offset=base_ap.offset + part_start * pitch + free_off,
                       ap=[[pitch, nparts]] + [list(d) for d in free_dims])

    nc = bacc.Bacc("TRN2", target_bir_lowering=False)
    G = nb // 8            # groups of 8 images
    NU = nb // 4           # units of 4 images

    XR = nc.declare_dram_parameter("xr", [128, NU * ROWL], f16, isOutput=False)
    dparams = {}
    for name, shape, dt in [("W1BLK", [128, 128], f16), ("W2DY", [96, 192], f16),
                            ("B2R", [128, 1], f32),
                            ("W3P", [128, 768], f16), ("B3", [128, 1], f32),
                            ("W1FC", [128, 16 * 512], f16), ("B1FC", [1, 512], f32),
                            ("W2FC", [128, 64], f32), ("B2FC", [16, 1], f32),
                            ("URTI", [16, 48], f32),
                            ("WH", [16, 128], f32), ("AH", [128, 1], f32),
                            ("CH", [128, 1], f32),
                            ("H2WT", [128, 100], f32), ("H2B", [1, 100], f32)]:
        dparams[name] = nc.declare_dram_parameter(name, shape, dt, isOutput=False)
    OUT = nc.declare_dram_parameter("out", [nb, 100], f32, isOutput=True)

    R2N = 8 * 324 + 16     # 8 image planes of 18x18 + slack
    R3N = 8 * 100 + 8

    with tile.TileContext(nc) as tc, ExitStack() as ctx:
        const = ctx.enter_context(tc.tile_pool(name="const", bufs=1))
        ct = {}

        def load_param(name, eng):
            t = const.tile(list(dparams[name].shape), dparams[name].dtype,
                           tag="c_" + name)
            eng.dma_start(t[:], dparams[name][:])
            ct[name] = t

        r1p = ctx.enter_context(tc.tile_pool(name="r1p", bufs=6))
        r1t = [None] * G

        def load_r1(g):
            # two per-unit DMAs so conv1 on unit 0 starts before unit 1 lands
            r1 = r1p.tile([128, 2 * ROWL], f16, tag="r1")
            for u in range(2):
                nc.sync.dma_start(r1[:, u * ROWL:(u + 1) * ROWL],
                                  XR[:, (g * 2 + u) * ROWL:(g * 2 + u + 1) * ROWL])
            r1t[g] = r1

        # critical-path DMAs on SP, in consumption order
        load_param("W1BLK", nc.sync)
        load_r1(0)
        load_r1(1)
        load_param("W2DY", nc.sync)
        load_param("B2R", nc.sync)
        load_r1(2)
        load_param("W3P", nc.sync)
        load_param("B3", nc.sync)
        load_r1(3)

        ident = const.tile([128, 128], f32, tag="ident")
        make_identity(nc, ident[:])
        ones_r = const.tile([1, 128], f32, tag="ones_r")
        nc.vector.memset(ones_r[:], 1.0)
        ones_c = const.tile([16, 1], f32, tag="ones_c")
        nc.vector.memset(ones_c[:], 1.0)
        zp = const.tile([128, 1], f32, tag="zp")
        nc.vector.memset(zp[:], 0.0)
        ef = const.tile([128, nb * 16], f16, tag="ef")   # fc1 input accumulator

        # --- persistent triple-buffered conv input tiles with zero borders ---
        r2b, r3b = [], []
        for k in range(6):
            r2 = const.tile([128, R2N], f16, tag=f"r2_{k}")
            nc.gpsimd.memset(view(r2[:], 0, 32, 0, [[324, 8], [17 * 18, 2], [1, 18]]), 0.0)
            nc.gpsimd.memset(view(r2[:], 0, 32, 0, [[324, 8], [18, 18], [17, 2]]), 0.0)
            nc.gpsimd.memset(r2[0:32, 8 * 324:R2N], 0.0)
            r2b.append(r2)
            r3 = const.tile([128, R3N], f16, tag=f"r3_{k}")
            nc.gpsimd.memset(view(r3[:], 0, 64, 0, [[100, 8], [9 * 10, 2], [1, 10]]), 0.0)
            nc.gpsimd.memset(view(r3[:], 0, 64, 0, [[100, 8], [10, 10], [9, 2]]), 0.0)
            nc.gpsimd.memset(r3[0:64, 800:R3N], 0.0)
            r3b.append(r3)

        # --- PE p-state warmup: dummy matmuls while the first DMAs land ---
        with tc.tile_pool(name="wup", bufs=1) as wup, \
             tc.tile_pool(name="wps", bufs=1, space="PSUM") as wps:
            w0 = wup.tile([128, 512], f16, tag="w0")
            nc.vector.memset(w0[:], 0.0)
            pw = wps.tile([128, 512], f32, tag="pw")
            for _ in range(8):
                nc.tensor.matmul(pw[:], w0[:, 0:128], w0[:], start=True, stop=True)

        with tc.tile_pool(name="sxp", bufs=3) as sxp, \
             tc.tile_pool(name="ps1p", bufs=2, space="PSUM") as ps1p, \
             tc.tile_pool(name="ps2p", bufs=2, space="PSUM") as ps2p, \
             tc.tile_pool(name="ps3p", bufs=2, space="PSUM") as ps3p:

            def stage1(g):
                """conv1 + pool1 for group g (8 images, 2 units)."""
                if g + 4 < G:
                    load_r1(g + 4)
                r1 = r1t[g]
                r2 = r2b[g % 6]
                for u in range(2):
                    ps1 = ps1p.tile([128, 1024], f32, tag="ps1")
                    for h in range(2):
                        rhs = view(r1[:], 0, 128, u * ROWL + h * 16 * 34,
                                   [[34, 16], [1, 32]])
                        nc.tensor.matmul(ps1[:, h * 512:(h + 1) * 512],
                                         ct['W1BLK'][:], rhs, start=True, stop=True)
                    # pool1: unit 0 evacuates via Act relu + DVE fp16 max tree,
                    # unit 1 via a direct DVE 4:1 max reduce (relu rides the
                    # per-image write's max-with-0); both end in t1 [128, 256]
                    t1 = sxp.tile([128, 256], f16, tag="t1")
                    if u == 0:
                        s1 = sxp.tile([128, 1024], f16, tag="s1")
                        nc.scalar.activation(s1[:], ps1[:], AF.Relu)
                        th = sxp.tile([128, 512], f16, tag="th")
                        nc.vector.tensor_tensor(
                            th[:], view(s1[:], 0, 128, 0, [[32, 32], [2, 16]]),
                            view(s1[:], 0, 128, 1, [[32, 32], [2, 16]]), op=ALU.max)
                        nc.vector.tensor_tensor(
                            t1[:], view(th[:], 0, 128, 0, [[32, 16], [1, 16]]),
                            view(th[:], 0, 128, 16, [[32, 16], [1, 16]]), op=ALU.max)
                    else:
                        nc.vector.tensor_reduce(
                            out=t1[:],
                            in_=view(ps1[:], 0, 128, 0,
                                     [[64, 16], [2, 16], [32, 2], [1, 2]]),
                            op=ALU.max, axis=AX.XY)
                    for j in range(4):
                        iu = 4 * u + j
                        dstv = view(r2[:], 0, 32, iu * 324 + 19, [[18, 16], [1, 16]])
                        srcv = view(t1[:], 32 * j, 32, 0, [[16, 16], [1, 16]])
                        if j % 2 == 0:
                            nc.vector.tensor_scalar(dstv, srcv, zp[32 * j:32 * j + 32, :],
                                                    0.0, op0=ALU.add, op1=ALU.max)
                        else:
                            nc.gpsimd.tensor_scalar(dstv, srcv, zp[32 * j:32 * j + 32, :],
                                                    0.0, op0=ALU.add, op1=ALU.max)
                # dx-shift copies for conv2's K-packing (parts 32:96)
                pitch = r2[:].ap[0][0]
                for dx in (1, 2):
                    src = bass.AP(tensor=r2[:].tensor, offset=r2[:].offset + dx,
                                  ap=[[pitch, 32], [1, 8 * 324 + 4 - dx]])
                    eng = nc.gpsimd if dx == 1 else nc.scalar
                    eng.dma_start(r2[32 * dx:32 * dx + 32, 0:8 * 324 + 4 - dx],
                                  src)

            def stage2(g):
                """conv2 + pool2 + conv3 + avgpool for group g."""
                r2 = r2b[g % 6]
                r3 = r3b[g % 6]
                for v in range(2):
                    ps2 = ps2p.tile([128, 512], f32, tag="ps2")
                    for i in range(4):            # image 4v+i within group
                        iu = 4 * v + i
                        pb, st = i % 2, i // 2    # partition block, subtile
                        for dy in range(3):
                            rhs = view(r2[:], 0, 96, iu * 324 + dy * 18,
                                       [[18, 16], [1, 16]])
                            nc.tensor.matmul(
                                ps2[64 * pb:64 * pb + 64, st * 256:st * 256 + 256],
                                ct['W2DY'][:, dy * 64:(dy + 1) * 64], rhs,
                                start=(dy == 0), stop=(dy == 2),
                                tile_position=(0, 64 * pb))
                    # pool2: tile 0 via Act bias+relu + DVE fp16 max tree,
                    # tile 1 via direct DVE reduce (bias+relu ride the write)
                    t2 = sxp.tile([128, 128], f16, tag="t2")
                    if v == 0:
                        s2 = sxp.tile([128, 512], f16, tag="s2")
                        nc.scalar.activation(s2[:], ps2[:], AF.Relu,
                                             bias=ct['B2R'][:], scale=1.0)
                        th2 = sxp.tile([128, 256], f16, tag="th2")
                        nc.vector.tensor_tensor(
                            th2[:], view(s2[:], 0, 128, 0, [[256, 2], [16, 16], [2, 8]]),
                            view(s2[:], 0, 128, 1, [[256, 2], [16, 16], [2, 8]]),
                            op=ALU.max)
                        nc.vector.tensor_tensor(
                            t2[:], view(th2[:], 0, 128, 0, [[128, 2], [16, 8], [1, 8]]),
                            view(th2[:], 0, 128, 8, [[128, 2], [16, 8], [1, 8]]),
                            op=ALU.max)
                    else:
                        nc.vector.tensor_reduce(
                            out=t2[:],
                            in_=view(ps2[:], 0, 128, 0,
                                     [[256, 2], [32, 8], [2, 8], [16, 2], [1, 2]]),
                            op=ALU.max, axis=AX.XY)
                    for i in range(4):
                        iu = 4 * v + i
                        pb, st = i % 2, i // 2
                        dstv = view(r3[:], 0, 64, iu * 100 + 11, [[10, 8], [1, 8]])
                        srcv = view(t2[:], 64 * pb, 64, st * 64, [[8, 8], [1, 8]])
                        scal = (zp if v == 0 else ct['B2R'])[64 * pb:64 * pb + 64, :]
                        if i % 2 == 0:
                            nc.vector.tensor_scalar(dstv, srcv, scal, 0.0,
                                                    op0=ALU.add, op1=ALU.max)
                        else:
                            nc.gpsimd.tensor_scalar(dstv, srcv, scal, 0.0,
                                                    op0=ALU.add, op1=ALU.max)
                # dx-shift for conv3 (parts 64:128 = planes shifted by 1)
                src = bass.AP(tensor=r3[:].tensor, offset=r3[:].offset + 1,
                              ap=[[r3[:].ap[0][0], 64], [1, 800]])
                nc.gpsimd.dma_start(r3[64:128, 0:800], src)

                # conv3: 6 accumulating passes, full 512-wide stream
                ps3 = ps3p.tile([128, 512], f32, tag="ps3")
                for dy in range(3):
                    rhsA = view(r3[:], 0, 128, dy * 10, [[100, 8], [10, 8], [1, 8]])
                    nc.tensor.matmul(ps3[:], ct['W3P'][:, (2 * dy) * 128:(2 * dy + 1) * 128],
                                     rhsA, start=(dy == 0), stop=False)
                    rhsB = view(r3[:], 0, 64, dy * 10 + 2, [[100, 8], [10, 8], [1, 8]])
                    nc.tensor.matmul(ps3[:], ct['W3P'][0:64, (2 * dy + 1) * 128:(2 * dy + 2) * 128],
                                     rhsB, start=False, stop=(dy == 2))
                # relu+bias then avgpool adds into ef (0.25 folded into W1FC)
                h3 = sxp.tile([128, 512], f16, tag="h3")
                nc.scalar.activation(h3[:], ps3[:], AF.Relu, bias=ct['B3'][:],
                                     scale=1.0)
                with nc.allow_low_precision("avgpool sums 4 values; fp16 is fine"):
                    e1 = sxp.tile([128, 256], f16, tag="e1")
                    nc.vector.tensor_tensor(
                        e1[:], view(h3[:], 0, 128, 0, [[64, 8], [8, 8], [2, 4]]),
                        view(h3[:], 0, 128, 1, [[64, 8], [8, 8], [2, 4]]), op=ALU.add)
                    efd = view(ef[:], 0, 128, g * 128, [[16, 8], [4, 4], [1, 4]])
                    nc.vector.tensor_tensor(
                        efd, view(e1[:], 0, 128, 0, [[32, 8], [8, 4], [1, 4]]),
                        view(e1[:], 0, 128, 4, [[32, 8], [8, 4], [1, 4]]), op=ALU.add)

            for g in range(G):
                stage1(g)
                if g >= 4:
                    stage2(g - 4)
            for g in range(G - 4, G):
                stage2(g)

        # tail-only params: big W1FC via gpsimd SWDGE (cheap engine-side),
        # the small ones on Act
        load_param("W1FC", nc.gpsimd)
        for name in ["B1FC", "W2FC", "B2FC", "URTI", "WH",
                     "AH", "CH", "H2WT", "H2B"]:
            load_param(name, nc.scalar)

        # ------------------- tail: fc1 / fc2 / quantum / head -------------------
        with tc.tile_pool(name="tsb", bufs=1) as tsb, \
             tc.tile_pool(name="psfp", bufs=1, space="PSUM") as psfp, \
             tc.tile_pool(name="pstp", bufs=2, space="PSUM") as pstp, \
             tc.tile_pool(name="tp1", bufs=2, space="PSUM") as tp1, \
             tc.tile_pool(name="tp2", bufs=2, space="PSUM") as tp2:

            psf = psfp.tile([nb, 512], f32, tag="psf")
            for s in range(16):
                lhsT = view(ef[:], 0, 128, s, [[16, nb]])
                nc.tensor.matmul(psf[:], lhsT,
                                 ct['W1FC'][:, s * 512:(s + 1) * 512],
                                 start=(s == 0), stop=False)
            nc.tensor.matmul(psf[:], ones_r[0:1, 0:nb], ct['B1FC'][:],
                             start=False, stop=True)
            h1t = tsb.tile([nb, 512], f32, tag="h1t")
            nc.scalar.activation(h1t[:], psf[:], AF.Relu)

            h1 = tsb.tile([128, 4 * nb], f32, tag="h1")
            for t in range(4):
                pst = pstp.tile([128, nb], f32, tag="pst")
                nc.tensor.transpose(pst[:], h1t[:, t * 128:(t + 1) * 128], ident[0:nb, 0:nb])
                nc.vector.tensor_copy(h1[:, t * nb:(t + 1) * nb], pst[:])

            psz = tp1.tile([16, nb], f32, tag="tp1")
            for t in range(4):
                nc.tensor.matmul(psz[:], ct['W2FC'][:, t * 16:(t + 1) * 16],
                                 h1[:, t * nb:(t + 1) * nb],
                                 start=(t == 0), stop=(t == 3))
            e = tsb.tile([16, nb], f32, tag="e")
            nc.scalar.activation(e[:], psz[:], AF.Exp, bias=ct['B2FC'][:], scale=1.0)

            # two parallel branches off e:
            #   A: |U e|^2 (statevector, unnormalized), B: 1/||e||^2 broadcast
            # (U unitary => ||U e|| = ||e||, so B comes straight from e)
            psru = tp2.tile([48, nb], f32, tag="tp2")
            nc.tensor.matmul(psru[:], ct['URTI'][:], e[:], start=True, stop=True)
            e2 = tsb.tile([16, nb], f32, tag="e2")
            nc.vector.tensor_mul(e2[:], e[:], e[:])
            pss = tp1.tile([1, nb], f32, tag="tp1")
            nc.tensor.matmul(pss[:], ones_c[:], e2[:], start=True, stop=True)
            tr = tsb.tile([16, nb], f32, tag="tr")
            nc.scalar.square(tr[:], psru[0:16, :])
            ti = tsb.tile([16, nb], f32, tag="ti")
            nc.scalar.square(ti[:], psru[32:48, :])
            rec = tsb.tile([1, nb], f32, tag="rec")
            nc.vector.reciprocal(rec[:], pss[:])
            pun = tsb.tile([16, nb], f32, tag="pun")
            nc.vector.tensor_add(pun[:], tr[:], ti[:])
            psq = tp2.tile([128, nb], f32, tag="tp2")
            nc.tensor.matmul(psq[:], ct['WH'][:], pun[:], start=True, stop=True)
            psb = tp1.tile([128, nb], f32, tag="tp1")
            nc.tensor.matmul(psb[:], ones_r[0:1, 0:128], rec[:], start=True, stop=True)
            bcs = tsb.tile([128, nb], f32, tag="bcs")
            nc.vector.tensor_copy(bcs[:], psb[:])
            hm2 = tsb.tile([128, nb], f32, tag="hm2")
            nc.vector.tensor_mul(hm2[:], psq[:], bcs[:])
            h2 = tsb.tile([128, nb], f32, tag="h2")
            nc.scalar.activation(h2[:], hm2[:], AF.Relu, bias=ct['CH'][:], scale=ct['AH'][:])

            pso = tp1.tile([nb, 100], f32, tag="tp1")
            nc.tensor.matmul(pso[:], h2[:], ct['H2WT'][:], start=True, stop=False)
            nc.tensor.matmul(pso[:], ones_r[0:1, 0:nb], ct['H2B'][:],
                             start=False, stop=True)
            outs = tsb.tile([nb, 100], f32, tag="outs")
            nc.scalar.copy(outs[:], pso[:])
            nc.sync.dma_start(OUT[:], outs[:])

    nc.finalize()
    return nc


def get_program(nb=NB):
    key = ("prog", nb)
    if key not in _cache:
        _cache[key] = _build_program(nb)
    return _cache[key]


# ---------------------------------------------------------------------------
# entry point
# ---------------------------------------------------------------------------

def kernel(**inputs):
    from concourse.bass_utils import run_bass_kernel_spmd

    x = np.asarray(inputs['x'], np.float32)
    B = x.shape[0]
    nb = B // NCORES
    hw = _host_weights({k: np.asarray(v) for k, v in inputs.items()})

    nc = get_program(nb)
    in_maps = []
    for c in range(NCORES):
        m = {'xr': _build_xr(x[c * nb:(c + 1) * nb])}
        m.update(hw)
        in_maps.append(m)
    res = run_bass_kernel_spmd(nc, in_maps, core_ids=list(range(NCORES)))
    return np.concatenate([res.results[c]['out'] for c in range(NCORES)], axis=0)


# revision 49
# speedup vs baseline: 1.3544x; 1.0233x over previous
"""Trainium2 Bass kernel for nn_EnhancedHybridModel.

Pipeline per core (pure data parallel over batch, 128 images/core):
  conv1(3->32,3x3,p1)+BN+ReLU -> maxpool2 -> conv2(32->64)+BN+ReLU -> maxpool2
  -> conv3(64->128)+BN+ReLU -> avgpool2 -> fc 2048->512 -> fc 512->16
  -> softmax -> 4-qubit statevector sim (collapses to two fixed real 16x16
  matmuls built on host from q_weights) -> head 4->128->100.

Layout decisions (matmul time on this target scales with the output free
size only, so every matmul packs as many of the 128 output partitions as
the layer allows):
  * conv1: 4 images block-diagonally packed on the PE (K = 4x(27+bias row),
    M = 4img x 32ch), two F=512 matmuls per 4-image unit.
  * maxpool: per group, half the PSUM evacuates through Act (relu/bias
    fused) followed by a DVE fp16 tensor_tensor max tree, half through a
    direct DVE 4:1 max tensor_reduce; per-image tensor_scalar writes
    (DVE 4x-mode / Pool ptr-form, relu+bias fused via max-with-0) scatter
    the pooled planes into the next conv's padded input tile.
  * conv2: 3 dy passes, K=96 dx-preshifted planes (dx shifts via one
    gpsimd-DGE and one Act-DGE SBUF-to-SBUF DMA per group).
  * conv3: 6 passes (3dy x {2dx-packed 128, dx2 64}); relu+bias via one Act
    pass, avgpool via fp16 adds with the 0.25 folded into fc1 weights.
  * 4-group software-pipeline skew between conv1 and conv2/conv3 keeps the
    PE dense; a dummy-matmul warmup covers the first input DMA so the PE
    p-state ramp completes early.
  * tail: fc2 softmax exp, then the quantum layer as one [16,48] matmul of
    [Re(U).T|Im(U).T]; unitarity gives the normalizer directly from
    sum(e^2), so the statevector and normalizer branches run concurrently.
"""

import numpy as np

NB = 128          # images per core
NCORES = 8
ROWL = 1160       # padded im2col plane length per image (1156 + slack)
EPS = 1e-5

_cache = {}


# ---------------------------------------------------------------------------
# host-side math (quantum layer constants, weight folding, im2col planes)
# ---------------------------------------------------------------------------

def _cnot_ring_matrix():
    M = np.zeros((16, 16), dtype=np.complex64)
    for b in range(16):
        bb = b
        for cw, tw in [(0, 1), (1, 2), (2, 3), (3, 0)]:
            if (bb >> (3 - cw)) & 1:
                bb ^= 1 << (3 - tw)
        M[bb, b] = 1.0
    return M


def _zsigns():
    return np.array([[1.0 - 2.0 * ((b >> (3 - w)) & 1) for b in range(16)]
                     for w in range(4)], dtype=np.float32)


def _quantum_unitary(q_weights):
    CN = _cnot_ring_matrix()
    U_tot = np.eye(16, dtype=np.complex64)
    for l in range(2):
        c = np.cos(q_weights[l] * 0.5).astype(np.complex64)
        s = np.sin(q_weights[l] * 0.5).astype(np.complex64)
        U = np.ones((1, 1), dtype=np.complex64)
        for q in range(4):
            g = np.array([[c[q], -1j * s[q]], [-1j * s[q], c[q]]], dtype=np.complex64)
            U = np.kron(U, g)
        U_tot = (CN @ U) @ U_tot
    return U_tot  # psi_out = psi_in @ U_tot.T


def _host_weights(inp):
    f32, f16 = np.float32, np.float16
    sc = f32(1.0 / np.sqrt(1.0 + EPS))
    out = {}

    # conv1: 4-image block-diagonal [128, 128]; row 32j+27 carries the bias.
    g1 = inp['bn1_g'] * sc
    w1 = np.zeros((28, 32), f32)
    for dy in range(3):
        for dx in range(3):
            for ci in range(3):
                w1[(dy * 3 + dx) * 3 + ci, :] = inp['conv1_w'][:, ci, dy, dx] * g1
    w1[27, :] = inp['conv1_b'] * g1 + inp['bn1_b']
    w1blk = np.zeros((128, 128), f16)
    for j in range(4):
        w1blk[32 * j:32 * j + 28, 32 * j:32 * j + 32] = w1
    out['W1BLK'] = w1blk

    # conv2: [96, 3*64] rows dx*32+ci; bias applied at the pool write stage.
    g2 = inp['bn2_g'] * sc
    w2 = np.zeros((96, 192), f32)
    for dy in range(3):
        for dx in range(3):
            for ci in range(32):
                w2[dx * 32 + ci, dy * 64:(dy + 1) * 64] = inp['conv2_w'][:, ci, dy, dx] * g2
    out['W2DY'] = w2.astype(f16)
    out['B2R'] = np.tile(inp['conv2_b'] * g2 + inp['bn2_b'], 2)[:, None].astype(f32)

    # conv3: 6 passes [128, 6*128]: pass 2*dy   = chunkA (dx0,dx1 on 128 rows)
    #                               pass 2*dy+1 = chunkB (dx2 on 64 rows)
    g3 = inp['bn3_g'] * sc
    w3 = np.zeros((128, 6 * 128), f32)
    for dy in range(3):
        for ci in range(64):
            for dx in range(2):
                w3[dx * 64 + ci, (2 * dy) * 128:(2 * dy + 1) * 128] = \
                    inp['conv3_w'][:, ci, dy, dx] * g3
            w3[ci, (2 * dy + 1) * 128:(2 * dy + 2) * 128] = \
                inp['conv3_w'][:, ci, dy, 2] * g3
    out['W3P'] = w3.astype(f16)
    out['B3'] = (inp['conv3_b'] * g3 + inp['bn3_b']).astype(f32)[:, None]

    # fc1 with avgpool folded in: input index = c*16 + s, pool = 0.25*sum
    fr1 = inp['fr1_w'].reshape(512, 128, 16)  # [m, c, s]
    w1fc = np.zeros((128, 16 * 512), f32)
    for s in range(16):
        w1fc[:, s * 512:(s + 1) * 512] = (fr1[:, :, s].T * 0.25)
    out['W1FC'] = w1fc.astype(f16)
    out['B1FCC'] = np.ascontiguousarray(
        inp['fr1_b'].reshape(4, 128).T.astype(f32))

    fr2 = inp['fr2_w']  # [16, 512]
    w2fc = np.zeros((128, 64), f32)
    for t in range(4):
        w2fc[:, t * 16:(t + 1) * 16] = fr2[:, t * 128:(t + 1) * 128].T
    out['W2FC'] = w2fc
    out['B2FC'] = inp['fr2_b'].astype(f32)[:, None]

    U = _quantum_unitary(np.asarray(inp['q_weights'], np.float64))
    # stacked [Re(U) | Im(U)] so one matmul produces both statevector parts
    urti = np.zeros((16, 48), f32)
    urti[:, 0:16] = np.real(U).T
    urti[:, 32:48] = np.imag(U).T   # at col 32 so Act reads a 32-aligned slice
    out['URTI'] = urti

    ZS = _zsigns()
    out['WH'] = np.ascontiguousarray((inp['h1_w'] @ ZS).T.astype(f32))  # [16, 128]
    ah = inp['bnh_g'] * sc
    out['AH'] = ah.astype(f32)[:, None]
    out['CH'] = (ah * inp['h1_b'] + inp['bnh_b']).astype(f32)[:, None]

    out['H2WT'] = np.ascontiguousarray(inp['h2_w'].T.astype(f32))  # [128, 100]
    out['H2B'] = inp['h2_b'].astype(f32)[None, :]
    return out


def _build_xr(x):
    """Host im2col planes for conv1, packed [128, (B//4)*ROWL]: partition
    32*j + r holds plane row r of image 4u+j at columns [u*ROWL, ...): the
    27 shifted flat padded planes, a ones row (r=27) for the bias, zeros
    above."""
    B = x.shape[0]
    U = B // 4
    xp = np.zeros((B, 3, 34 * 34 + 72), np.float16)  # flat plane + shift slack
    xpv = xp[:, :, :34 * 34].reshape(B, 3, 34, 34)
    xpv[:, :, 1:33, 1:33] = x
    xr = np.zeros((32, B, ROWL), np.float16)
    for dy in range(3):
        for dx in range(3):
            sh = dy * 34 + dx
            for ci in range(3):
                r = (dy * 3 + dx) * 3 + ci
                xr[r, :, :1156] = xp[:, ci, sh:sh + 1156]
    xr[27, :, :] = 1.0
    # [32, B, ROWL] -> [32, U, 4, ROWL] -> [4, 32, U, ROWL] -> [128, U*ROWL]
    xrb = xr.reshape(32, U, 4, ROWL).transpose(2, 0, 1, 3)
    return np.ascontiguousarray(xrb.reshape(128, U * ROWL))


# ---------------------------------------------------------------------------
# device program
# ---------------------------------------------------------------------------

def _build_program(nb):
    import concourse.bass as bass
    import concourse.tile as tile
    from concourse import bacc, mybir
    from concourse.masks import make_identity
    from contextlib import ExitStack

    f32 = mybir.dt.float32
    f16 = mybir.dt.float16
    AF = mybir.ActivationFunctionType
    ALU = mybir.AluOpType
    AX = mybir.AxisListType

    def view(base_ap, part_start, nparts, free_off, free_dims):
        pitch = base_ap.ap[0][0]
        return bass.AP(tensor=base_ap.tensor,
                       offset=base_ap.offset + part_start * pitch + free_off,
                       ap=[[pitch, nparts]] + [list(d) for d in free_dims])

    nc = bacc.Bacc("TRN2", target_bir_lowering=False)
    G = nb // 8            # groups of 8 images
    NU = nb // 4           # units of 4 images

    XR = nc.declare_dram_parameter("xr", [128, NU * ROWL], f16, isOutput=False)
    dparams = {}
    for name, shape, dt in [("W1BLK", [128, 128], f16), ("W2DY", [96, 192], f16),
                            ("B2R", [128, 1], f32),
                            ("W3P", [128, 768], f16), ("B3", [128, 1], f32),
                            ("W1FC", [128, 16 * 512], f16), ("B1FCC", [128, 4], f32),
                            ("W2FC", [128, 64], f32), ("B2FC", [16, 1], f32),
                            ("URTI", [16, 48], f32),
                            ("WH", [16, 128], f32), ("AH", [128, 1], f32),
                            ("CH", [128, 1], f32),
                            ("H2WT", [128, 100], f32), ("H2B", [1, 100], f32)]:
        dparams[name] = nc.declare_dram_parameter(name, shape, dt, isOutput=False)
    OUT = nc.declare_dram_parameter("out", [nb, 100], f32, isOutput=True)

    R2N = 8 * 324 + 16     # 8 image planes of 18x18 + slack
    R3N = 8 * 100 + 8

    with tile.TileContext(nc) as tc, ExitStack() as ctx:
        const = ctx.enter_context(tc.tile_pool(name="const", bufs=1))
        ct = {}

        def load_param(name, eng):
            t = const.tile(list(dparams[name].shape), dparams[name].dtype,
                           tag="c_" + name)
            eng.dma_start(t[:], dparams[name][:])
            ct[name] = t

        r1p = ctx.enter_context(tc.tile_pool(name="r1p", bufs=6))
        r1t = [None] * G

        def load_r1(g):
            # two per-unit DMAs so conv1 on unit 0 starts before unit 1 lands
            r1 = r1p.tile([128, 2 * ROWL], f16, tag="r1")
            for u in range(2):
                nc.sync.dma_start(r1[:, u * ROWL:(u + 1) * ROWL],
                                  XR[:, (g * 2 + u) * ROWL:(g * 2 + u + 1) * ROWL])
            r1t[g] = r1

        # critical-path DMAs on SP, in consumption order
        load_param("W1BLK", nc.sync)
        load_r1(0)
        load_r1(1)
        load_param("W2DY", nc.sync)
        load_param("B2R", nc.sync)
        load_r1(2)
        load_param("W3P", nc.sync)
        load_param("B3", nc.sync)
        load_r1(3)

        ident = const.tile([128, 128], f32, tag="ident")
        make_identity(nc, ident[:])
        ones_r = const.tile([1, 128], f32, tag="ones_r")
        nc.vector.memset(ones_r[:], 1.0)
        ones_c = const.tile([16, 1], f32, tag="ones_c")
        nc.vector.memset(ones_c[:], 1.0)
        zp = const.tile([128, 1], f32, tag="zp")
        nc.vector.memset(zp[:], 0.0)
        ef = const.tile([128, nb * 16], f16, tag="ef")   # fc1 input accumulator

        # --- persistent triple-buffered conv input tiles with zero borders ---
        r2b, r3b = [], []
        for k in range(6):
            r2 = const.tile([128, R2N], f16, tag=f"r2_{k}")
            nc.gpsimd.memset(view(r2[:], 0, 32, 0, [[324, 8], [17 * 18, 2], [1, 18]]), 0.0)
            nc.gpsimd.memset(view(r2[:], 0, 32, 0, [[324, 8], [18, 18], [17, 2]]), 0.0)
            nc.gpsimd.memset(r2[0:32, 8 * 324:R2N], 0.0)
            r2b.append(r2)
            r3 = const.tile([128, R3N], f16, tag=f"r3_{k}")
            nc.gpsimd.memset(view(r3[:], 0, 64, 0, [[100, 8], [9 * 10, 2], [1, 10]]), 0.0)
            nc.gpsimd.memset(view(r3[:], 0, 64, 0, [[100, 8], [10, 10], [9, 2]]), 0.0)
            nc.gpsimd.memset(r3[0:64, 800:R3N], 0.0)
            r3b.append(r3)

        # --- PE p-state warmup: dummy matmuls while the first DMAs land ---
        with tc.tile_pool(name="wup", bufs=1) as wup, \
             tc.tile_pool(name="wps", bufs=1, space="PSUM") as wps:
            w0 = wup.tile([128, 512], f16, tag="w0")
            nc.vector.memset(w0[:], 0.0)
            pw = wps.tile([128, 512], f32, tag="pw")
            for _ in range(8):
                nc.tensor.matmul(pw[:], w0[:, 0:128], w0[:], start=True, stop=True)

        with tc.tile_pool(name="sxp", bufs=3) as sxp, \
             tc.tile_pool(name="ps1p", bufs=2, space="PSUM") as ps1p, \
             tc.tile_pool(name="ps2p", bufs=2, space="PSUM") as ps2p, \
             tc.tile_pool(name="ps3p", bufs=2, space="PSUM") as ps3p:

            def stage1(g):
                """conv1 + pool1 for group g (8 images, 2 units)."""
                if g + 4 < G:
                    load_r1(g + 4)
                r1 = r1t[g]
                r2 = r2b[g % 6]
                for u in range(2):
                    ps1 = ps1p.tile([128, 1024], f32, tag="ps1")
                    for h in range(2):
                        rhs = view(r1[:], 0, 128, u * ROWL + h * 16 * 34,
                                   [[34, 16], [1, 32]])
                        nc.tensor.matmul(ps1[:, h * 512:(h + 1) * 512],
                                         ct['W1BLK'][:], rhs, start=True, stop=True)
                    # pool1: unit 0 evacuates via Act relu + DVE fp16 max tree,
                    # unit 1 via a direct DVE 4:1 max reduce (relu rides the
                    # per-image write's max-with-0); both end in t1 [128, 256]
                    t1 = sxp.tile([128, 256], f16, tag="t1")
                    if u == 0:
                        s1 = sxp.tile([128, 1024], f16, tag="s1")
                        nc.scalar.activation(s1[:], ps1[:], AF.Relu)
                        th = sxp.tile([128, 512], f16, tag="th")
                        nc.vector.tensor_tensor(
                            th[:], view(s1[:], 0, 128, 0, [[32, 32], [2, 16]]),
                            view(s1[:], 0, 128, 1, [[32, 32], [2, 16]]), op=ALU.max)
                        nc.vector.tensor_tensor(
                            t1[:], view(th[:], 0, 128, 0, [[32, 16], [1, 16]]),
                            view(th[:], 0, 128, 16, [[32, 16], [1, 16]]), op=ALU.max)
                    else:
                        nc.vector.tensor_reduce(
                            out=t1[:],
                            in_=view(ps1[:], 0, 128, 0,
                                     [[64, 16], [2, 16], [32, 2], [1, 2]]),
                            op=ALU.max, axis=AX.XY)
                    for j in range(4):
                        iu = 4 * u + j
                        dstv = view(r2[:], 0, 32, iu * 324 + 19, [[18, 16], [1, 16]])
                        srcv = view(t1[:], 32 * j, 32, 0, [[16, 16], [1, 16]])
                        if j % 2 == 0:
                            nc.vector.tensor_scalar(dstv, srcv, zp[32 * j:32 * j + 32, :],
                                                    0.0, op0=ALU.add, op1=ALU.max)
                        else:
                            nc.gpsimd.tensor_scalar(dstv, srcv, zp[32 * j:32 * j + 32, :],
                                                    0.0, op0=ALU.add, op1=ALU.max)
                # dx-shift copies for conv2's K-packing (parts 32:96)
                pitch = r2[:].ap[0][0]
                for dx in (1, 2):
                    src = bass.AP(tensor=r2[:].tensor, offset=r2[:].offset + dx,
                                  ap=[[pitch, 32], [1, 8 * 324 + 4 - dx]])
                    eng = nc.gpsimd if dx == 1 else nc.scalar
                    eng.dma_start(r2[32 * dx:32 * dx + 32, 0:8 * 324 + 4 - dx],
                                  src)

            def stage2(g):
                """conv2 + pool2 + conv3 + avgpool for group g."""
                r2 = r2b[g % 6]
                r3 = r3b[g % 6]
                for v in range(2):
                    ps2 = ps2p.tile([128, 512], f32, tag="ps2")
                    for i in range(4):            # image 4v+i within group
                        iu = 4 * v + i
                        pb, st = i % 2, i // 2    # partition block, subtile
                        for dy in range(3):
                            rhs = view(r2[:], 0, 96, iu * 324 + dy * 18,
                                       [[18, 16], [1, 16]])
                            nc.tensor.matmul(
                                ps2[64 * pb:64 * pb + 64, st * 256:st * 256 + 256],
                                ct['W2DY'][:, dy * 64:(dy + 1) * 64], rhs,
                                start=(dy == 0), stop=(dy == 2),
                                tile_position=(0, 64 * pb))
                    # pool2: tile 0 via Act bias+relu + DVE fp16 max tree,
                    # tile 1 via direct DVE reduce (bias+relu ride the write)
                    t2 = sxp.tile([128, 128], f16, tag="t2")
                    if v == 0:
                        s2 = sxp.tile([128, 512], f16, tag="s2")
                        nc.scalar.activation(s2[:], ps2[:], AF.Relu,
                                             bias=ct['B2R'][:], scale=1.0)
                        th2 = sxp.tile([128, 256], f16, tag="th2")
                        nc.vector.tensor_tensor(
                            th2[:], view(s2[:], 0, 128, 0, [[256, 2], [16, 16], [2, 8]]),
                            view(s2[:], 0, 128, 1, [[256, 2], [16, 16], [2, 8]]),
                            op=ALU.max)
                        nc.vector.tensor_tensor(
                            t2[:], view(th2[:], 0, 128, 0, [[128, 2], [16, 8], [1, 8]]),
                            view(th2[:], 0, 128, 8, [[128, 2], [16, 8], [1, 8]]),
                            op=ALU.max)
                    else:
                        nc.vector.tensor_reduce(
                            out=t2[:],
                            in_=view(ps2[:], 0, 128, 0,
                                     [[256, 2], [32, 8], [2, 8], [16, 2], [1, 2]]),
                            op=ALU.max, axis=AX.XY)
                    for i in range(4):
                        iu = 4 * v + i
                        pb, st = i % 2, i // 2
                        dstv = view(r3[:], 0, 64, iu * 100 + 11, [[10, 8], [1, 8]])
                        srcv = view(t2[:], 64 * pb, 64, st * 64, [[8, 8], [1, 8]])
                        scal = (zp if v == 0 else ct['B2R'])[64 * pb:64 * pb + 64, :]
                        if i % 2 == 0:
                            nc.vector.tensor_scalar(dstv, srcv, scal, 0.0,
                                                    op0=ALU.add, op1=ALU.max)
                        else:
                            nc.gpsimd.tensor_scalar(dstv, srcv, scal, 0.0,
                                                    op0=ALU.add, op1=ALU.max)
                # dx-shift for conv3 (parts 64:128 = planes shifted by 1)
                src = bass.AP(tensor=r3[:].tensor, offset=r3[:].offset + 1,
                              ap=[[r3[:].ap[0][0], 64], [1, 800]])
                nc.gpsimd.dma_start(r3[64:128, 0:800], src)

                # conv3: 6 accumulating passes, full 512-wide stream
                ps3 = ps3p.tile([128, 512], f32, tag="ps3")
                for dy in range(3):
                    rhsA = view(r3[:], 0, 128, dy * 10, [[100, 8], [10, 8], [1, 8]])
                    nc.tensor.matmul(ps3[:], ct['W3P'][:, (2 * dy) * 128:(2 * dy + 1) * 128],
                                     rhsA, start=(dy == 0), stop=False)
                    rhsB = view(r3[:], 0, 64, dy * 10 + 2, [[100, 8], [10, 8], [1, 8]])
                    nc.tensor.matmul(ps3[:], ct['W3P'][0:64, (2 * dy + 1) * 128:(2 * dy + 2) * 128],
                                     rhsB, start=False, stop=(dy == 2))
                # relu+bias then avgpool adds into ef (0.25 folded into W1FC)
                h3 = sxp.tile([128, 512], f16, tag="h3")
                nc.scalar.activation(h3[:], ps3[:], AF.Relu, bias=ct['B3'][:],
                                     scale=1.0)
                with nc.allow_low_precision("avgpool sums 4 values; fp16 is fine"):
                    e1 = sxp.tile([128, 256], f16, tag="e1")
                    nc.vector.tensor_tensor(
                        e1[:], view(h3[:], 0, 128, 0, [[64, 8], [8, 8], [2, 4]]),
                        view(h3[:], 0, 128, 1, [[64, 8], [8, 8], [2, 4]]), op=ALU.add)
                    efd = view(ef[:], 0, 128, g * 128, [[16, 8], [4, 4], [1, 4]])
                    nc.vector.tensor_tensor(
                        efd, view(e1[:], 0, 128, 0, [[32, 8], [8, 4], [1, 4]]),
                        view(e1[:], 0, 128, 4, [[32, 8], [8, 4], [1, 4]]), op=ALU.add)

            for g in range(G):
                stage1(g)
                if g >= 4:
                    stage2(g - 4)
            for g in range(G - 4, G):
                stage2(g)

        # tail-only params: big W1FC via gpsimd SWDGE (cheap engine-side),
        # the small ones on Act
        load_param("W1FC", nc.gpsimd)
        for name in ["B1FCC", "W2FC", "B2FC", "URTI", "WH",
                     "AH", "CH", "H2WT", "H2B"]:
            load_param(name, nc.scalar)

        # ------------------- tail: fc1 / fc2 / quantum / head -------------------
        with tc.tile_pool(name="tsb", bufs=1) as tsb, \
             tc.tile_pool(name="psfp", bufs=2, space="PSUM") as psfp, \
             tc.tile_pool(name="pstp", bufs=2, space="PSUM") as pstp, \
             tc.tile_pool(name="tp1", bufs=2, space="PSUM") as tp1, \
             tc.tile_pool(name="tp2", bufs=2, space="PSUM") as tp2:

            # fc1 transposed: feature chunks on partitions, images on the
            # free dim; each chunk's relu+bias overlaps the next chunk's
            # matmuls and no PE transposes are needed for fc2
            h1 = tsb.tile([128, 4 * nb], f32, tag="h1")
            for t in range(4):
                psf = psfp.tile([128, nb], f32, tag="psf")
                for s in range(16):
                    rhs = view(ef[:], 0, 128, s, [[16, nb]])
                    nc.tensor.matmul(psf[:],
                                     ct['W1FC'][:, s * 512 + t * 128:s * 512 + t * 128 + 128],
                                     rhs, start=(s == 0), stop=(s == 15))
                nc.scalar.activation(h1[:, t * nb:(t + 1) * nb], psf[:], AF.Relu,
                                     bias=ct['B1FCC'][:, t:t + 1], scale=1.0)

            psz = tp1.tile([16, nb], f32, tag="tp1")
            for t in range(4):
                nc.tensor.matmul(psz[:], ct['W2FC'][:, t * 16:(t + 1) * 16],
                                 h1[:, t * nb:(t + 1) * nb],
                                 start=(t == 0), stop=(t == 3))
            e = tsb.tile([16, nb], f32, tag="e")
            nc.scalar.activation(e[:], psz[:], AF.Exp, bias=ct['B2FC'][:], scale=1.0)

            # two parallel branches off e:
            #   A: |U e|^2 (statevector, unnormalized), B: 1/||e||^2 broadcast
            # (U unitary => ||U e|| = ||e||, so B comes straight from e)
            psru = tp2.tile([48, nb], f32, tag="tp2")
            nc.tensor.matmul(psru[:], ct['URTI'][:], e[:], start=True, stop=True)
            e2 = tsb.tile([16, nb], f32, tag="e2")
            nc.vector.tensor_mul(e2[:], e[:], e[:])
            pss = tp1.tile([1, nb], f32, tag="tp1")
            nc.tensor.matmul(pss[:], ones_c[:], e2[:], start=True, stop=True)
            tr = tsb.tile([16, nb], f32, tag="tr")
            nc.scalar.square(tr[:], psru[0:16, :])
            ti = tsb.tile([16, nb], f32, tag="ti")
            nc.scalar.square(ti[:], psru[32:48, :])
            rec = tsb.tile([1, nb], f32, tag="rec")
            nc.vector.reciprocal(rec[:], pss[:])
            pun = tsb.tile([16, nb], f32, tag="pun")
            nc.vector.tensor_add(pun[:], tr[:], ti[:])
            psq = tp2.tile([128, nb], f32, tag="tp2")
            nc.tensor.matmul(psq[:], ct['WH'][:], pun[:], start=True, stop=True)
            psb = tp1.tile([128, nb], f32, tag="tp1")
            nc.tensor.matmul(psb[:], ones_r[0:1, 0:128], rec[:], start=True, stop=True)
            bcs = tsb.tile([128, nb], f32, tag="bcs")
            nc.vector.tensor_copy(bcs[:], psb[:])
            hm2 = tsb.tile([128, nb], f32, tag="hm2")
            nc.vector.tensor_mul(hm2[:], psq[:], bcs[:])
            h2 = tsb.tile([128, nb], f32, tag="h2")
            nc.scalar.activation(h2[:], hm2[:], AF.Relu, bias=ct['CH'][:], scale=ct['AH'][:])

            pso = tp1.tile([nb, 100], f32, tag="tp1")
            nc.tensor.matmul(pso[:], h2[:], ct['H2WT'][:], start=True, stop=False)
            nc.tensor.matmul(pso[:], ones_r[0:1, 0:nb], ct['H2B'][:],
                             start=False, stop=True)
            outs = tsb.tile([nb, 100], f32, tag="outs")
            nc.scalar.copy(outs[:], pso[:])
            nc.sync.dma_start(OUT[:], outs[:])

    nc.finalize()
    return nc


def get_program(nb=NB):
    key = ("prog", nb)
    if key not in _cache:
        _cache[key] = _build_program(nb)
    return _cache[key]


# ---------------------------------------------------------------------------
# entry point
# ---------------------------------------------------------------------------

def kernel(**inputs):
    from concourse.bass_utils import run_bass_kernel_spmd

    x = np.asarray(inputs['x'], np.float32)
    B = x.shape[0]
    nb = B // NCORES
    hw = _host_weights({k: np.asarray(v) for k, v in inputs.items()})

    nc = get_program(nb)
    in_maps = []
    for c in range(NCORES):
        m = {'xr': _build_xr(x[c * nb:(c + 1) * nb])}
        m.update(hw)
        in_maps.append(m)
    res = run_bass_kernel_spmd(nc, in_maps, core_ids=list(range(NCORES)))
    return np.concatenate([res.results[c]['out'] for c in range(NCORES)], axis=0)
